# revision 6
# baseline (speedup 1.0000x reference)
"""ColorNorm Trainium2 kernel.

Problem: per-sample 3x3 color-matching solve over N=1024*1024 pixels.
  A = src[b] (3,N), B = dst[b] (3,N)
  AAt = Ac@Ac.T + 1e-3 I ; BAt = Bc@Ac.T ; x = BAt@inv(AAt)
  out[b] = x@Ac + Bmean
Sharding: data-parallel over batch (16 samples -> 8 cores x 2 samples).

Per-core pipeline (fp16 data plane, fp32 accumulation/solve/output):
  load:  A,B cast fp32->fp16 in-flight by SWDGE DMA; A stays resident.
  pass1: DVE tensor_mul (fp16 2x) computes the 12 cross products; their
         reduction runs on PE as ones-matmuls accumulating into col-tiled
         PSUM [1,512] partials (finished by a tiny partition_all_reduce).
         ScalarE Square+accum does the 3 diagonals; DVE tensor_scalar+accum
         (4x fp16) does raw channel sums; one ones-matmul reduces those
         across partitions.
  solve: 3x3 inverse via adjugate (tiny fp32 DVE ops on partition 0).
  pass2: out_i = sum_j x_ij*A_j + d_i via fp16 PE matmuls with diag(x_ij)
         stationary weights accumulating in fp32 PSUM; ScalarE evicts with
         the +d_i bias fused; 1MiB fp32 store DMAs.
"""

import sys

for _p in ("/opt/trn_rl_repo", "/opt/pypackages"):
    if _p not in sys.path:
        sys.path.append(_p)

from contextlib import ExitStack

import numpy as np

import concourse.bacc as bacc
import concourse.bass as bass
import concourse.tile as tile
from concourse import bass_isa, masks, mybir
from concourse._compat import with_exitstack

# ---- hardcoded problem geometry (per core) ----
B_CORE = 2          # samples per core
C = 3               # channels
H = W = 1024
N = H * W           # 1048576 pixels per channel
P = 128             # SBUF partitions
F = N // P          # 8192 free elems per partition per channel
Q = 2048            # quarter-chunk free size
NQ = F // Q         # 4 quarters
HB = 4096           # B half-channel free size
MM = 512            # matmul free-dim chunk (one PSUM bank)
NCORES = 8
RIDGE = 1e-3
import os
GP_N = int(os.environ.get("CN_GP_N", "0"))
B_BUFS = int(os.environ.get("CN_B_BUFS", "4"))
PS_BUFS = int(os.environ.get("CN_PS_BUFS", "3"))
PE_RED = int(os.environ.get("CN_PE_RED", "1"))
PTW = int(os.environ.get("CN_PTW", "512"))
SCRB_BUFS = int(os.environ.get("CN_SCRB", "4"))
A_BUFS = int(os.environ.get("CN_A_BUFS", "6"))

F32 = mybir.dt.float32
F16 = mybir.dt.float16
ALU = mybir.AluOpType
ACTF = mybir.ActivationFunctionType

# 6 unique AA pairs; symmetric index map
A_PAIRS = [(0, 0), (0, 1), (0, 2), (1, 1), (1, 2), (2, 2)]
SYM = {(0, 0): 0, (0, 1): 1, (0, 2): 2, (1, 1): 3, (1, 2): 4, (2, 2): 5}


def _rd(ap, dims):
    """Rebuild an AP keeping its partition dim, replacing free dims."""
    return bass.AP(ap.tensor, ap.offset, [ap.ap[0]] + dims)


@with_exitstack
def _colornorm(ctx: ExitStack, tc: "tile.TileContext", src, dst, out):
    nc = tc.nc
    srcv = src.rearrange("b c (p q) w -> b c p (q w)", p=P)  # [2,3,128,8192]
    dstv = dst.rearrange("b c (p q) w -> b c p (q w)", p=P)
    outv = out.rearrange("b c (p q) w -> b c p (q w)", p=P)

    singles = ctx.enter_context(tc.tile_pool(name="singles", bufs=1))
    a_pool = ctx.enter_context(tc.tile_pool(name="a_pool", bufs=A_BUFS))
    b_pool = ctx.enter_context(tc.tile_pool(name="b_pool", bufs=B_BUFS))
    scr_pool = ctx.enter_context(tc.tile_pool(name="scr", bufs=1))
    scrb_pool = ctx.enter_context(tc.tile_pool(name="scrb", bufs=SCRB_BUFS))
    gscr_pool = ctx.enter_context(tc.tile_pool(name="gscr", bufs=1))
    ascr_pool = ctx.enter_context(tc.tile_pool(name="ascr", bufs=1))
    acc_pool = ctx.enter_context(tc.tile_pool(name="accs", bufs=2))
    solve_pool = ctx.enter_context(tc.tile_pool(name="solve", bufs=2))
    dg_pool = ctx.enter_context(tc.tile_pool(name="dg", bufs=1))
    stage_pool = ctx.enter_context(tc.tile_pool(name="stage", bufs=2))
    ps_stat = ctx.enter_context(tc.tile_pool(name="ps_stat", bufs=2, space="PSUM"))
    ps_acc = ctx.enter_context(tc.tile_pool(name="ps_acc", bufs=3, space="PSUM"))
    ps_out = ctx.enter_context(tc.tile_pool(name="ps_out", bufs=PS_BUFS, space="PSUM"))

    ones = singles.tile([P, 1], F32)
    nc.vector.memset(ones, 1.0)
    ones16 = singles.tile([P, 1], F16)
    nc.vector.memset(ones16, 1.0)
    eye = singles.tile([P, P], F16)
    masks.make_identity(nc, eye[:])

    for s in range(B_CORE):
        # ------------- load (fp32 -> fp16 cast in DMA) -------------
        a_t = [a_pool.tile([P, F], F16, tag="ach", name="ach") for _ in range(C)]
        for c in range(C):
            nc.gpsimd.dma_start(out=a_t[c][:], in_=srcv[s, c])
        b_t = [[None, None] for _ in range(C)]
        for c in range(C):
            for h in range(2):
                b_t[c][h] = b_pool.tile([P, HB], F16, tag="bh", name="bh")
                nc.gpsimd.dma_start(out=b_t[c][h][:],
                                    in_=dstv[s, c][:, h * HB:(h + 1) * HB])

        def aq(c, q):  # quarter view of A channel
            return a_t[c][:, q * Q:(q + 1) * Q]

        def bh(c, h):  # half view of B
            return b_t[c][h][:]

        # merged per-partition accumulator columns:
        #  0-2   A cross pairs (0,1),(0,2),(1,2)   [full channel]
        #  3-5   A diag (ScalarE Square)           [full channel]
        #  6-23  BA pairs (c,j) x half h: 6+2*(3c+j)+h
        #  24-26 raw A sums [full channel]
        #  27-32 raw B sums (c,h): 27+2c+h
        acc = acc_pool.tile([P, 33], F32, tag="acc", name="acc")
        A_CROSS = [(0, 1), (0, 2), (1, 2)]
        # BA pairs whose multiply runs on GpSimd (idle otherwise)
        GP_BA = set([1, 3, 5, 7, 0, 8][:GP_N])

        def prod(col, x_ap, y_ap, width, pool_mul=False):
            # fp16 TT runs 2x, single-src ts+accum runs 4x; the fused
            # 2-input STT would be 1x — two fast ops beat one slow one.
            # pool_mul pushes the multiply to GpSimd (library TensorTensor).
            if pool_mul:
                scr = gscr_pool.tile([P, HB], F16, tag="gscr", name="gscr")
                nc.gpsimd.tensor_mul(out=scr[:, 0:width], in0=x_ap, in1=y_ap)
            else:
                scr = scr_pool.tile([P, F], F16, tag="scr", name="scr")
                nc.vector.tensor_mul(out=scr[:, 0:width], in0=x_ap, in1=y_ap)
            nc.vector.tensor_scalar(
                out=scr[:, 0:width], in0=scr[:, 0:width], scalar1=1.0,
                scalar2=0.0, op0=ALU.mult, op1=ALU.add,
                accum_out=acc[:, col:col + 1])

        # PE-reduce variant state: pair k -> psum bank k//4, col-group k%4
        psa = [ps_acc.tile([P, MM], F32, tag="psa", name="psa")
               for _ in range(3)] if PE_RED else None
        pair_mm_seen = [0] * 12
        PAIR_MMS = [16] * 3 + [16] * 9  # total 512-col mms per pair

        def prod_pe(k, x_ap, y_ap, width):
            """multiply on DVE; reduce via ones-matmul into psum[1,512]
            at partition 32*(k%4) of bank k//4 (PSUM accumulation).
            Separate scratch pool: PE is the consumer, so these tiles
            must not throttle DVE's next multiply."""
            scr = scrb_pool.tile([P, HB], F16, tag="scrb", name="scrb")
            nc.vector.tensor_mul(out=scr[:, 0:width], in0=x_ap, in1=y_ap)
            bank, grp = k // 4, k % 4
            for m in range(width // MM):
                first = pair_mm_seen[k] == 0
                pair_mm_seen[k] += 1
                last = pair_mm_seen[k] == PAIR_MMS[k]
                nc.tensor.matmul(
                    psa[bank][32 * grp:32 * grp + 1, :], ones16[:],
                    scr[:, m * MM:(m + 1) * MM],
                    start=first, stop=last,
                    tile_position=(0, 32 * grp))

        def raw_sum(col, x_ap, width):
            scr = scr_pool.tile([P, F], F16, tag="scr", name="scr")
            nc.vector.tensor_scalar(
                out=scr[:, 0:width], in0=x_ap, scalar1=1.0, scalar2=0.0,
                op0=ALU.mult, op1=ALU.add,
                accum_out=acc[:, col:col + 1])

        # A-channel stats (full-channel ops)
        asq = acc_pool.tile([P, 6], F32, tag="asq", name="asq")
        for c in range(C):
            raw_sum(24 + c, a_t[c][:], F)
            for h in range(2):
                ascr = ascr_pool.tile([P, HB], F16, tag="ascr", name="ascr")
                nc.scalar.activation(
                    out=ascr[:], in_=a_t[c][:, h * HB:(h + 1) * HB],
                    func=ACTF.Square, accum_out=asq[:, 2 * c + h: 2 * c + h + 1])
        # fold the square halves into acc cols 3..5 (tiny)
        nc.vector.reduce_sum(out=acc[:, 3:6].rearrange("p (c o) -> p c o", o=1),
                             in_=asq[:, 0:6].rearrange("p (c h) -> p c h", h=2),
                             axis=mybir.AxisListType.X)
        for k, (i, j) in enumerate(A_CROSS):
            prod(k, a_t[i][:], a_t[j][:], F)
        # B stats (half-channel granularity to pipeline with B loads)
        for c in range(C):
            for h in range(2):
                raw_sum(27 + 2 * c + h, bh(c, h), HB)
                for j in range(C):
                    if PE_RED:
                        prod_pe(3 + 3 * c + j, bh(c, h),
                                a_t[j][:, h * HB:(h + 1) * HB], HB)
                    else:
                        prod(6 + 2 * (3 * c + j) + h,
                             bh(c, h), a_t[j][:, h * HB:(h + 1) * HB], HB,
                             pool_mul=(3 * c + j) in GP_BA)

        # cross-partition reduce on PE: ones.T @ acc -> [1, 33]
        pst = ps_stat.tile([1, 40], F32, tag="pst", name="pst")
        nc.tensor.matmul(pst[0:1, 0:33], ones[:], acc[:],
                         start=True, stop=True)
        stats = solve_pool.tile([1, 40], F32, tag="stats", name="stats")
        nc.vector.tensor_copy(out=stats[0:1, 0:33], in_=pst[0:1, 0:33])
        BA9 = solve_pool.tile([1, 9], F32, tag="BA9", name="BA9")
        if PE_RED:
            # pair partials live at [1,512] psum regions; reduce each to a
            # scalar at its partition, then all-reduce across partitions
            prow = solve_pool.tile([P, 12], F32, tag="prow", name="prow")
            nc.vector.memset(prow[:], 0.0)
            for k in range(12):
                bank, grp = k // 4, k % 4
                nc.vector.reduce_sum(
                    out=prow[32 * grp:32 * grp + 1, k:k + 1],
                    in_=psa[bank][32 * grp:32 * grp + 1, :],
                    axis=mybir.AxisListType.X)
            prow2 = solve_pool.tile([P, 12], F32, tag="prow2", name="prow2")
            nc.gpsimd.partition_all_reduce(
                prow2[:], prow[:], channels=P,
                reduce_op=bass_isa.ReduceOp.add)
            nc.vector.tensor_copy(out=BA9[:], in_=prow2[0:1, 3:12])
        else:
            # BA pair sums: collapse the two half partials
            nc.vector.reduce_sum(out=BA9[:], axis=mybir.AxisListType.X,
                                 in_=stats[0:1, 6:24].rearrange(
                                     "p (k h) -> p k h", h=2))

        # ---------------- 3x3 solve on partition 0 ----------------
        sumB = solve_pool.tile([1, 3], F32, tag="sumB", name="sumB")
        nc.vector.reduce_sum(out=sumB[:], axis=mybir.AxisListType.X,
                             in_=stats[0:1, 27:33].rearrange(
                                 "p (c h) -> p c h", h=2))
        Am = solve_pool.tile([1, 3], F32, tag="Am", name="Am")
        Bm = solve_pool.tile([1, 3], F32, tag="Bm", name="Bm")
        nc.vector.tensor_scalar_mul(out=Am[:], in0=stats[0:1, 24:27],
                                    scalar1=1.0 / N)
        nc.vector.tensor_scalar_mul(out=Bm[:], in0=sumB[:], scalar1=1.0 / N)

        AA9 = solve_pool.tile([1, 9], F32, tag="AA9", name="AA9")
        SYM3 = {(0, 1): 0, (0, 2): 1, (1, 2): 2}
        for i in range(C):
            for j in range(C):
                col = 3 + i if i == j else SYM3[(min(i, j), max(i, j))]
                nc.vector.tensor_copy(out=AA9[0:1, 3 * i + j: 3 * i + j + 1],
                                      in_=stats[0:1, col:col + 1])

        # centered: AAc = AA - N*Am Am^T (+ridge); BAc = BA - N*Bm Am^T
        outer = solve_pool.tile([1, 9], F32, tag="outer", name="outer")
        o3x3 = outer[0:1, :].rearrange("p (i j) -> p i j", j=3)
        nc.vector.tensor_mul(out=o3x3, in0=_rd(Am[0:1, 0:1], [[1, 3], [0, 3]]),
                             in1=_rd(Am[0:1, 0:1], [[0, 3], [1, 3]]))
        AAc = solve_pool.tile([1, 9], F32, tag="AAc", name="AAc")
        nc.vector.scalar_tensor_tensor(out=AAc[:], in0=outer[:],
                                       scalar=-float(N), in1=AA9[:],
                                       op0=ALU.mult, op1=ALU.add)
        dg_ap = _rd(AAc[0:1, 0:1], [[4, 3]])
        nc.vector.tensor_scalar_add(out=dg_ap, in0=dg_ap, scalar1=RIDGE)
        nc.vector.tensor_mul(out=o3x3, in0=_rd(Bm[0:1, 0:1], [[1, 3], [0, 3]]),
                             in1=_rd(Am[0:1, 0:1], [[0, 3], [1, 3]]))
        BAc = solve_pool.tile([1, 9], F32, tag="BAc", name="BAc")
        nc.vector.scalar_tensor_tensor(out=BAc[:], in0=outer[:],
                                       scalar=-float(N), in1=BA9[:],
                                       op0=ALU.mult, op1=ALU.add)

        # inverse via adjugate: M2 = 6x6 tiling of AAc (mod-3 access)
        M2 = solve_pool.tile([1, 36], F32, tag="M2", name="M2")
        for dr in (0, 3):
            for dc in (0, 3):
                nc.vector.tensor_copy(
                    out=_rd(M2[0:1, 6 * dr + dc: 6 * dr + dc + 1],
                            [[6, 3], [1, 3]]),
                    in_=AAc[0:1, :].rearrange("p (i j) -> p i j", j=3))
        t1 = solve_pool.tile([1, 9], F32, tag="t1", name="t1")
        t2 = solve_pool.tile([1, 9], F32, tag="t2", name="t2")
        nc.vector.tensor_mul(out=t1[0:1, :].rearrange("p (i j) -> p i j", j=3),
                             in0=_rd(M2[0:1, 7:8], [[6, 3], [1, 3]]),
                             in1=_rd(M2[0:1, 14:15], [[6, 3], [1, 3]]))
        nc.vector.tensor_mul(out=t2[0:1, :].rearrange("p (i j) -> p i j", j=3),
                             in0=_rd(M2[0:1, 8:9], [[6, 3], [1, 3]]),
                             in1=_rd(M2[0:1, 13:14], [[6, 3], [1, 3]]))
        cof = solve_pool.tile([1, 9], F32, tag="cof", name="cof")
        nc.vector.tensor_sub(out=cof[:], in0=t1[:], in1=t2[:])

        det = solve_pool.tile([1, 1], F32, tag="det", name="det")
        dscr = solve_pool.tile([1, 3], F32, tag="dscr", name="dscr")
        nc.vector.scalar_tensor_tensor(
            out=dscr[:], in0=AAc[0:1, 0:3], scalar=1.0, in1=cof[0:1, 0:3],
            op0=ALU.mult, op1=ALU.mult, accum_out=det[:])
        rdet = solve_pool.tile([1, 1], F32, tag="rdet", name="rdet")
        nc.vector.reciprocal(out=rdet[:], in_=det[:])

        inv9 = solve_pool.tile([1, 9], F32, tag="inv9", name="inv9")
        nc.vector.tensor_scalar_mul(
            out=inv9[0:1, :].rearrange("p (i j) -> p i j", j=3),
            in0=_rd(cof[0:1, 0:1], [[1, 3], [3, 3]]),  # cof^T
            scalar1=rdet[:])

        # x = BAc @ inv  (tmp27[i,k,j] = BAc[i,j]*inv[j,k], reduce j)
        tmp27 = solve_pool.tile([1, 27], F32, tag="tmp27", name="tmp27")
        nc.vector.tensor_mul(
            out=tmp27[0:1, :].rearrange("p (i k j) -> p i k j", k=3, j=3),
            in0=_rd(BAc[0:1, 0:1], [[3, 3], [0, 3], [1, 3]]),
            in1=_rd(inv9[0:1, 0:1], [[0, 3], [1, 3], [3, 3]]))
        x9 = solve_pool.tile([1, 9], F32, tag="x9", name="x9")
        nc.vector.reduce_sum(
            out=x9[0:1, :].rearrange("p (i k) -> p i k", k=3),
            in_=tmp27[0:1, :].rearrange("p (i k j) -> p i k j", k=3, j=3),
            axis=mybir.AxisListType.X)

        # d = Bm - x@Am
        tmp9 = solve_pool.tile([1, 9], F32, tag="tmp9", name="tmp9")
        nc.vector.tensor_mul(
            out=tmp9[0:1, :].rearrange("p (i j) -> p i j", j=3),
            in0=x9[0:1, :].rearrange("p (i j) -> p i j", j=3),
            in1=_rd(Am[0:1, 0:1], [[0, 3], [1, 3]]))
        xAm = solve_pool.tile([1, 3], F32, tag="xAm", name="xAm")
        nc.vector.reduce_sum(out=xAm[:], axis=mybir.AxisListType.X,
                             in_=tmp9[0:1, :].rearrange("p (i j) -> p i j", j=3))
        sol = solve_pool.tile([1, 12], F32, tag="sol", name="sol")
        nc.vector.tensor_copy(out=sol[0:1, 0:9], in_=x9[:])
        nc.vector.tensor_sub(out=sol[0:1, 9:12], in0=Bm[:], in1=xAm[:])

        # broadcast x,d to all partitions
        xb = solve_pool.tile([P, 12], F32, tag="xb", name="xb")
        nc.gpsimd.partition_broadcast(xb[:], sol[0:1, 0:12])

        # diag(x_ij) fp16 weight tiles
        dg = [[dg_pool.tile([P, P], F16, tag=f"dg{i}{j}", name=f"dg{i}{j}")
               for j in range(C)] for i in range(C)]
        for i in range(C):
            for j in range(C):
                nc.vector.tensor_scalar_mul(
                    out=dg[i][j][:], in0=eye[:],
                    scalar1=xb[:, 3 * i + j: 3 * i + j + 1])

        # -------- pass 2: out_i = sum_j x_ij A_j + d_i --------
        # For the last sample the tail has no other work: DVE computes
        # channel 0 (ts+stt chain) while PE does channels 1,2.
        dve_ch = {0} if (PE_RED and s == B_CORE - 1) else set()
        for g in range(NQ):
            for i in range(C):
                if i in dve_ch:
                    stage = stage_pool.tile([P, Q], F32, tag="stage",
                                            name="stage")
                    ga = slice(g * Q, (g + 1) * Q)
                    nc.vector.tensor_scalar(
                        out=stage[:], in0=a_t[0][:, ga],
                        scalar1=xb[:, 3 * i: 3 * i + 1],
                        scalar2=xb[:, 9 + i: 10 + i],
                        op0=ALU.mult, op1=ALU.add)
                    for j in (1, 2):
                        nc.vector.scalar_tensor_tensor(
                            out=stage[:], in0=a_t[j][:, ga],
                            scalar=xb[:, 3 * i + j: 3 * i + j + 1],
                            in1=stage[:], op0=ALU.mult, op1=ALU.add)
                    nc.scalar.dma_start(out=outv[s, i][:, ga], in_=stage[:])
                    continue
                stage = stage_pool.tile([P, Q], F32, tag="stage", name="stage")
                for hh in range(Q // PTW):
                    pt = ps_out.tile([P, PTW], F32, tag="pt", name="pt")
                    for j in range(C):
                        for cc in range(PTW // MM):
                            o0 = hh * PTW + cc * MM
                            nc.tensor.matmul(
                                pt[:, cc * MM:(cc + 1) * MM], dg[i][j][:],
                                a_t[j][:, g * Q + o0: g * Q + o0 + MM],
                                start=(j == 0), stop=(j == 2))
                    nc.scalar.add(out=stage[:, hh * PTW:(hh + 1) * PTW],
                                  in_=pt[:], add=xb[:, 9 + i: 10 + i])
                nc.scalar.dma_start(out=outv[s, i][:, g * Q:(g + 1) * Q],
                                    in_=stage[:])


def build_nc() -> "bass.Bass":
    nc = bacc.Bacc("TRN2", target_bir_lowering=False)
    src = nc.dram_tensor("src", [B_CORE, C, H, W], F32, kind="ExternalInput")
    dst = nc.dram_tensor("dst", [B_CORE, C, H, W], F32, kind="ExternalInput")
    out = nc.dram_tensor("out", [B_CORE, C, H, W], F32, kind="ExternalOutput")
    with tile.TileContext(nc) as tc:
        _colornorm(tc, src[:], dst[:], out[:])
    nc.finalize()
    return nc


_NC = None


def _get_nc():
    global _NC
    if _NC is None:
        _NC = build_nc()
    return _NC


TRACE = False
LAST_RESULT = None  # BassKernelResults of the most recent run (for profiling)


def kernel(src, dst):
    from concourse.bass_utils import run_bass_kernel_spmd

    global LAST_RESULT
    src = np.ascontiguousarray(np.asarray(src, dtype=np.float32))
    dst = np.ascontiguousarray(np.asarray(dst, dtype=np.float32))
    assert src.shape == (NCORES * B_CORE, C, H, W), src.shape
    nc = _get_nc()
    in_maps = [
        {
            "src": np.ascontiguousarray(src[i * B_CORE:(i + 1) * B_CORE]),
            "dst": np.ascontiguousarray(dst[i * B_CORE:(i + 1) * B_CORE]),
        }
        for i in range(NCORES)
    ]
    res = run_bass_kernel_spmd(nc, in_maps, core_ids=list(range(NCORES)),
                               trace=TRACE)
    LAST_RESULT = res
    return np.concatenate([r["out"] for r in res.results], axis=0)



# revision 7
# speedup vs baseline: 1.7704x; 1.7704x over previous
"""ColorNorm Trainium2 kernel, v2: PE-Gram pass1.

Problem: per-sample 3x3 color-matching solve over N=1024*1024 pixels.
  A = src[b] (3,N), B = dst[b] (3,N)
  AAt = Ac@Ac.T + 1e-3 I ; BAt = Bc@Ac.T ; x = BAt@inv(AAt)
  out[b] = x@Ac + Bmean
Sharding: data-parallel over batch (16 samples -> 8 cores x 2 samples).

v2 design (cost-model driven):
  - fp16 data plane (cast in DMA); fp16 DRAM output (host upcasts), which
    halves store traffic on the serial DMA resource.
  - pass1 pair sums run mostly on PE as chunked Gram matmuls: for each
    128-col chunk q, psum[128,128] += W_chunk.T @ X_chunk accumulated over
    64 chunks; the Gram diagonal holds per-col-offset partials, extracted
    by one DVE tensor_tensor_reduce against an identity into acc columns.
    Raw channel sums ride along as 1-col ones matmuls sharing the loaded
    weights; a cross-partition ones-matmul finishes all stats at once.
  - a few pairs run on DVE (TT+TS) and one on GpSimd to balance engines.
  - the 3x3 solve runs on GpSimd so its long serial chain doesn't get
    head-of-line blocked behind the next sample's big DVE products.
  - pass2 on PE with diag(x_ij) stationary weights; ScalarE evicts with
    +d_i bias fused into an fp16 stage stored via SP HWDGE; a couple of
    tail quarters go through a DVE ts+stt path to shorten the tail.
"""

import os
import sys

for _p in ("/opt/trn_rl_repo", "/opt/pypackages"):
    if _p not in sys.path:
        sys.path.append(_p)

from contextlib import ExitStack

import numpy as np

import concourse.bacc as bacc
import concourse.bass as bass
import concourse.tile as tile
from concourse import bass_isa, masks, mybir
from concourse._compat import with_exitstack

# ---- hardcoded problem geometry (per core) ----
B_CORE = 2          # samples per core
C = 3               # channels
H = W = 1024
N = H * W           # 1048576 pixels per channel
P = 128             # SBUF partitions
F = N // P          # 8192 free elems per partition per channel
QW = 2048           # B load quarter width
NQ = F // QW        # 4 quarters
CHW = 128           # Gram chunk width (psum partition dim)
NCH = F // CHW      # 64 chunks
NCORES = 8
RIDGE = 1e-3

F32 = mybir.dt.float32
F16 = mybir.dt.float16
ALU = mybir.AluOpType

# knobs
N_BA_DVE = int(os.environ.get("CN_BA_DVE", "3"))   # BA pairs on DVE
N_BA_POOL = int(os.environ.get("CN_BA_POOL", "0"))  # BA pairs on GpSimd
N_P2_DVE = int(os.environ.get("CN_P2_DVE", "4"))   # s1 pass2 units on DVE
PS_OUT_W = int(os.environ.get("CN_PSW", "512"))    # pass2 psum tile width
ST_W = 2048                                        # store chunk width
GRAM_MAJOR_TAIL = int(os.environ.get("CN_GMT", "1"))
ILV = os.environ.get("CN_ILV", "qu")  # interleave pattern of s1-BA/pass2-s0

_ALL_BA = [(0, 0), (1, 1), (2, 2), (0, 1), (1, 0), (0, 2), (2, 0),
           (1, 2), (2, 1)]
N_BA_DVE_S = [int(x) for x in
              os.environ.get("CN_BA_DVE_S", "2,3").split(",")]
BA_DVE_PAIRS_S = [_ALL_BA[:n] for n in N_BA_DVE_S]
BA_POOL_PAIRS = []
BA_PE_PAIRS_S = [[p for p in _ALL_BA if p not in BA_DVE_PAIRS_S[s]]
                 for s in range(2)]
A_CROSS_PE = [(0, 1)]
A_CROSS = [(0, 2), (1, 2)]
SYM3 = {(0, 1): 0, (0, 2): 1, (1, 2): 2}

# acc column map (per-partition partials; stats-mm reduces partitions)
#   0-8   BA(c,j) at 3c+j ; 9-17 AA row-major (diag 9+4i, cross 9+3i+j for
#   i<j; lower triangle filled in the solve) ; 18-20 rawA ; 21-23 rawB
ACC_W = 24
# mstat region: cols of gram bank 2: rawA 384-386, rawB 387-389,
# stats-mm out row0 392-412
MST0 = 384
STAT0 = 392


def _rd(ap, dims):
    """Rebuild an AP keeping its partition dim, replacing free dims."""
    return bass.AP(ap.tensor, ap.offset, [ap.ap[0]] + dims)


@with_exitstack
def _colornorm(ctx: ExitStack, tc: "tile.TileContext", src, dst, out):
    nc = tc.nc
    srcv = src.rearrange("b c (p q) w -> b c p (q w)", p=P)  # [2,3,128,8192]
    dstv = dst.rearrange("b c (p q) w -> b c p (q w)", p=P)
    outv = out.rearrange("b c (p q) w -> b c p (q w)", p=P)

    singles = ctx.enter_context(tc.tile_pool(name="singles", bufs=1))
    a_pool = ctx.enter_context(tc.tile_pool(name="a_pool", bufs=6))
    b_pool = ctx.enter_context(tc.tile_pool(name="b_pool", bufs=14))
    scr_pool = ctx.enter_context(tc.tile_pool(name="scr", bufs=1))
    gscr_pool = ctx.enter_context(tc.tile_pool(name="gscr", bufs=1))
    ex_pool = ctx.enter_context(tc.tile_pool(name="exscr", bufs=2))
    acc_pool = ctx.enter_context(tc.tile_pool(name="accs", bufs=2))
    solve_pool = ctx.enter_context(tc.tile_pool(name="solve", bufs=2))
    dg_pool = ctx.enter_context(tc.tile_pool(name="dg", bufs=2))
    stage_pool = ctx.enter_context(tc.tile_pool(name="stage", bufs=3))
    pstage_pool = ctx.enter_context(tc.tile_pool(name="pstage", bufs=2))
    dstage_pool = ctx.enter_context(tc.tile_pool(name="dstage", bufs=2))
    ps_gram = ctx.enter_context(tc.tile_pool(name="ps_gram", bufs=6,
                                             space="PSUM"))
    ps_out = ctx.enter_context(tc.tile_pool(name="ps_out", bufs=2,
                                            space="PSUM"))

    ones16 = singles.tile([P, 1], F16)
    nc.vector.memset(ones16, 1.0)
    ones32 = singles.tile([P, 1], F32)
    nc.vector.memset(ones32, 1.0)
    one1 = singles.tile([1, 1], F32)
    nc.vector.memset(one1, 1.0)
    eye16 = singles.tile([P, P], F16)
    masks.make_identity(nc, eye16[:])
    eye32 = singles.tile([P, P], F32)
    masks.make_identity(nc, eye32[:])

    # per-sample state
    barrier_scr = [None]
    a_t = [None, None]
    b_t = [None, None]
    acc = [None, None]
    stat_t = [None, None]     # [s] -> [P, 512] psum tile holding stats row
    xb = [None, None]
    dg = [None, None]
    pending_ex = [[], []]     # [s] -> [(gram_tile, acc_col), ...]

    def emit_loads_a(s):
        a_t[s] = [a_pool.tile([P, F], F16, tag="ach", name="ach")
                  for _ in range(C)]
        for c in range(C):
            nc.gpsimd.dma_start(out=a_t[s][c][:], in_=srcv[s, c])
        if b_t[s] is None:
            b_t[s] = [[None] * NQ for _ in range(C)]

    def emit_loads_b(s, qq0, qq1, c_major=False):
        order = ([(c, qq) for c in range(C) for qq in range(qq0, qq1)]
                 if c_major else
                 [(c, qq) for qq in range(qq0, qq1) for c in range(C)])
        for c, qq in order:
            t = b_pool.tile([P, QW], F16, tag="bq", name="bq")
            b_t[s][c][qq] = t
            nc.gpsimd.dma_start(
                out=t[:], in_=dstv[s, c][:, qq * QW:(qq + 1) * QW])

    def emit_loads(s):
        emit_loads_a(s)
        emit_loads_b(s, 0, NQ, c_major=True)

    def emit_alloc_pass1(s):
        acc[s] = acc_pool.tile([P, ACC_W], F32, tag="acc", name="acc")
        nc.vector.memset(acc[s], 0.0)

    def a_chunk(s, c, q):
        return a_t[s][c][:, q * CHW:(q + 1) * CHW]

    def b_chunk(s, c, q):
        qq, lc = q // (QW // CHW), q % (QW // CHW)
        return b_t[s][c][qq][:, lc * CHW:(lc + 1) * CHW]

    def emit_gram(s, kind, x, y, raw=None):
        """One Gram = one whole rotating psum bank; PSUM start zeroes the
        full 2KB zero region per partition, so groups can never share a
        bank. Raw channel sums ride along in their own bank, sharing the
        loaded weights; ScalarE copies them out right after the stop."""
        gt = ps_gram.tile([P, 512], F32, tag="g", name="g")
        rt = ps_gram.tile([P, 512], F32, tag="g", name="g") if raw else None
        for q in range(NCH):
            st, sp = q == 0, q == NCH - 1
            lhs = (a_chunk(s, x, q) if kind in ("AAD", "AAX")
                   else b_chunk(s, x, q))
            nc.tensor.matmul(gt[:, 0:128], lhs, a_chunk(s, y, q),
                             start=st, stop=sp)
            if raw:
                nc.tensor.matmul(rt[:, 0:1], lhs, ones16[:],
                                 start=st, stop=sp)
        if raw:
            rkind, ch = raw
            col = (18 if rkind == "A" else 21) + ch
            nc.scalar.activation(out=acc[s][:, col:col + 1], in_=rt[:, 0:1],
                                 func=mybir.ActivationFunctionType.Copy)
        if kind == "BA":
            col = 3 * x + y
        else:
            col = 9 + (4 * x if kind == "AAD" else 3 * x + y)
        pending_ex[s].append((gt, col))

    def emit_pe_aad(s):
        # AA-diag + PE-cross grams + rawA; A-only so it fills DMA-gated PE
        # idle windows
        for i in range(C):
            emit_gram(s, "AAD", i, i, raw=("A", i))
        for (i, j) in A_CROSS_PE:
            emit_gram(s, "AAX", i, j)

    def ba_gram_list(s):
        return [(c, j) for c in range(C)
                for j in [j2 for (cc, j2) in BA_PE_PAIRS_S[s] if cc == c]]

    def emit_pe_ba_gram(s, c, j):
        js = [j2 for (cc, j2) in BA_PE_PAIRS_S[s] if cc == c]
        raw = ("B", c) if j == js[-1] else None
        emit_gram(s, "BA", c, j, raw=raw)

    def drain_extracts(s, n=None):
        take = pending_ex[s] if n is None else pending_ex[s][:n]
        pending_ex[s] = [] if n is None else pending_ex[s][n:]
        for gt, col in take:
            # tensor_tensor_reduce crashes the device at runtime, so the
            # diagonal extract is a TT eye-mask + TS accumulate instead
            ex = ex_pool.tile([P, P], F32, tag="ex", name="ex")
            nc.vector.tensor_mul(out=ex[:], in0=gt[:, 0:128], in1=eye32[:])
            nc.vector.tensor_scalar(
                out=ex[:], in0=ex[:], scalar1=1.0, scalar2=0.0,
                op0=ALU.mult, op1=ALU.add,
                accum_out=acc[s][:, col:col + 1])

    def emit_dve_products(s):
        # AA cross pairs: full-width TT + TS accum
        for k, (i, j) in enumerate(A_CROSS):
            drain_extracts(s, 3)
            if s == 1 and barrier_scr[0] is not None:
                scr = barrier_scr[0]
                barrier_scr[0] = None
            else:
                scr = scr_pool.tile([P, F], F16, tag="scr", name="scr")
            nc.vector.tensor_mul(out=scr[:], in0=a_t[s][i][:],
                                 in1=a_t[s][j][:])
            nc.vector.tensor_scalar(
                out=scr[:], in0=scr[:], scalar1=1.0, scalar2=0.0,
                op0=ALU.mult, op1=ALU.add,
                accum_out=acc[s][:, 9 + 3 * i + j:10 + 3 * i + j])
        # BA pairs on DVE / GpSimd: per-quarter TT, one DVE TS accum
        for (c, j) in BA_DVE_PAIRS_S[s] + BA_POOL_PAIRS:
            drain_extracts(s, 2)
            on_pool = (c, j) in BA_POOL_PAIRS
            pool_p = gscr_pool if on_pool else scr_pool
            eng = nc.gpsimd if on_pool else nc.vector
            scr = pool_p.tile([P, F], F16, tag="pscr" if on_pool else "scr",
                              name="pscr" if on_pool else "scr")
            for qq in range(NQ):
                eng.tensor_mul(
                    out=scr[:, qq * QW:(qq + 1) * QW], in0=b_t[s][c][qq][:],
                    in1=a_t[s][j][:, qq * QW:(qq + 1) * QW])
            nc.vector.tensor_scalar(
                out=scr[:], in0=scr[:], scalar1=1.0, scalar2=0.0,
                op0=ALU.mult, op1=ALU.add,
                accum_out=acc[s][:, 3 * c + j:3 * c + j + 1])

    def emit_extracts(s):
        drain_extracts(s)

    def emit_stats(s):
        stat_t[s] = ps_gram.tile([P, 512], F32, tag="g", name="g")
        nc.tensor.matmul(stat_t[s][0:1, 0:ACC_W], ones32[:],
                         acc[s][:], start=True, stop=True)

    def emit_solve(s):
        # solve on DVE. The serial chain must not share the engine with big
        # ops that are concurrently ready (they bypass stalled ops via the
        # exec window and every hop then waits behind a multi-us op), so
        # the next sample's DVE products are gated on a barrier tile that
        # the end of this solve writes (see emit_dve_products).
        v = nc.vector
        stats = solve_pool.tile([1, ACC_W], F32, tag="stats", name="stats")
        v.tensor_copy(out=stats[:], in_=stat_t[s][0:1, 0:ACC_W])

        # fill AA lower triangle from upper: cols (12,15)<-(10,11); 16<-14
        v.tensor_copy(out=_rd(stats[0:1, 12:13], [[3, 2]]),
                      in_=stats[0:1, 10:12])
        v.tensor_copy(out=stats[0:1, 16:17], in_=stats[0:1, 14:15])

        # means: one op over rawA|rawB
        Am6 = solve_pool.tile([1, 6], F32, tag="Am6", name="Am6")
        v.tensor_scalar_mul(out=Am6[:], in0=stats[0:1, 18:24],
                            scalar1=1.0 / N)
        Am = Am6[0:1, 0:3]
        Bm = Am6[0:1, 3:6]

        outer = solve_pool.tile([1, 9], F32, tag="outer", name="outer")
        o3x3 = outer[0:1, :].rearrange("p (i j) -> p i j", j=3)
        v.tensor_mul(out=o3x3, in0=_rd(Am[0:1, 0:1], [[1, 3], [0, 3]]),
                     in1=_rd(Am[0:1, 0:1], [[0, 3], [1, 3]]))
        AAc = solve_pool.tile([1, 9], F32, tag="AAc", name="AAc")
        v.scalar_tensor_tensor(out=AAc[:], in0=outer[:], scalar=-float(N),
                               in1=stats[0:1, 9:18], op0=ALU.mult,
                               op1=ALU.add)
        dg_ap = _rd(AAc[0:1, 0:1], [[4, 3]])
        v.tensor_scalar_add(out=dg_ap, in0=dg_ap, scalar1=RIDGE)
        v.tensor_mul(out=o3x3, in0=_rd(Bm[0:1, 0:1], [[1, 3], [0, 3]]),
                     in1=_rd(Am[0:1, 0:1], [[0, 3], [1, 3]]))
        BAc = solve_pool.tile([1, 9], F32, tag="BAc", name="BAc")
        v.scalar_tensor_tensor(out=BAc[:], in0=outer[:], scalar=-float(N),
                               in1=stats[0:1, 0:9], op0=ALU.mult,
                               op1=ALU.add)

        # M2 = 2x2 tiling of AAc in one 4-dim strided copy
        M2 = solve_pool.tile([1, 36], F32, tag="M2", name="M2")
        v.tensor_copy(
            out=_rd(M2[0:1, 0:1], [[18, 2], [3, 2], [6, 3], [1, 3]]),
            in_=_rd(AAc[0:1, 0:1], [[0, 2], [0, 2], [3, 3], [1, 3]]))
        t1 = solve_pool.tile([1, 9], F32, tag="t1", name="t1")
        t2 = solve_pool.tile([1, 9], F32, tag="t2", name="t2")
        v.tensor_mul(out=t1[0:1, :].rearrange("p (i j) -> p i j", j=3),
                     in0=_rd(M2[0:1, 7:8], [[6, 3], [1, 3]]),
                     in1=_rd(M2[0:1, 14:15], [[6, 3], [1, 3]]))
        v.tensor_mul(out=t2[0:1, :].rearrange("p (i j) -> p i j", j=3),
                     in0=_rd(M2[0:1, 8:9], [[6, 3], [1, 3]]),
                     in1=_rd(M2[0:1, 13:14], [[6, 3], [1, 3]]))
        cof = solve_pool.tile([1, 9], F32, tag="cof", name="cof")
        v.tensor_sub(out=cof[:], in0=t1[:], in1=t2[:])

        det = solve_pool.tile([1, 1], F32, tag="det", name="det")
        dscr = solve_pool.tile([1, 3], F32, tag="dscr", name="dscr")
        v.scalar_tensor_tensor(
            out=dscr[:], in0=AAc[0:1, 0:3], scalar=1.0, in1=cof[0:1, 0:3],
            op0=ALU.mult, op1=ALU.mult, accum_out=det[:])
        rdet = solve_pool.tile([1, 1], F32, tag="rdet", name="rdet")
        v.reciprocal(out=rdet[:], in_=det[:])

        inv9 = solve_pool.tile([1, 9], F32, tag="inv9", name="inv9")
        v.tensor_scalar_mul(
            out=inv9[0:1, :].rearrange("p (i j) -> p i j", j=3),
            in0=_rd(cof[0:1, 0:1], [[1, 3], [3, 3]]),  # cof^T
            scalar1=rdet[:])

        tmp27 = solve_pool.tile([1, 27], F32, tag="tmp27", name="tmp27")
        v.tensor_mul(
            out=tmp27[0:1, :].rearrange("p (i k j) -> p i k j", k=3, j=3),
            in0=_rd(BAc[0:1, 0:1], [[3, 3], [0, 3], [1, 3]]),
            in1=_rd(inv9[0:1, 0:1], [[0, 3], [1, 3], [3, 3]]))
        # x9 into sol[0:9]; d = Bm - x@Am into sol[9:12]
        sol = solve_pool.tile([1, 12], F32, tag="sol", name="sol")
        x9 = sol[0:1, 0:9]
        v.tensor_add(out=x9, in0=_rd(tmp27[0:1, 0:1], [[3, 9]]),
                     in1=_rd(tmp27[0:1, 1:2], [[3, 9]]))
        v.tensor_add(out=x9, in0=x9,
                     in1=_rd(tmp27[0:1, 2:3], [[3, 9]]))

        tmp9 = solve_pool.tile([1, 9], F32, tag="tmp9", name="tmp9")
        v.tensor_mul(
            out=tmp9[0:1, :].rearrange("p (i j) -> p i j", j=3),
            in0=x9.rearrange("p (i j) -> p i j", j=3),
            in1=_rd(Am[0:1, 0:1], [[0, 3], [1, 3]]))
        xAm = solve_pool.tile([1, 3], F32, tag="xAm", name="xAm")
        v.tensor_add(out=xAm[:], in0=_rd(tmp9[0:1, 0:1], [[3, 3]]),
                     in1=_rd(tmp9[0:1, 1:2], [[3, 3]]))
        v.tensor_add(out=xAm[:], in0=xAm[:],
                     in1=_rd(tmp9[0:1, 2:3], [[3, 3]]))
        v.tensor_sub(out=sol[0:1, 9:12], in0=Bm[:], in1=xAm[:])

        xb[s] = solve_pool.tile([P, 12], F32, tag="xb", name="xb")
        nc.gpsimd.partition_broadcast(xb[s][:], sol[0:1, 0:12])

        # all 9 diag(x_ij) weight tiles in one broadcast-AP tensor_mul
        xb16 = solve_pool.tile([P, 12], F16, tag="xb16", name="xb16")
        v.tensor_copy(out=xb16[:], in_=xb[s][:])
        dgall = dg_pool.tile([P, 9 * P], F16, tag="dgall", name="dgall")
        v.tensor_mul(out=_rd(dgall[:, 0:1], [[128, 9], [1, 128]]),
                     in0=_rd(eye16[:, 0:1], [[0, 9], [1, 128]]),
                     in1=_rd(xb16[:, 0:1], [[1, 9], [0, 128]]))
        dg[s] = [[dgall[:, (3 * i + j) * P:(3 * i + j + 1) * P]
                  for j in range(C)] for i in range(C)]
        if s == 0:
            # barrier tile: sample 1's first DVE product writes this tile,
            # so it cannot start (or bypass-run) before the solve finishes
            bar = scr_pool.tile([P, F], F16, tag="scr", name="scr")
            v.tensor_copy(out=bar[0:1, 0:1], in_=dgall[0:1, 0:1])
            barrier_scr[0] = bar

    def emit_pass2_unit(s, g, i, eng=None):
        if eng is None:
            stage = stage_pool.tile([P, ST_W], F16, tag="stage", name="stage")
        elif eng is nc.vector:
            stage = dstage_pool.tile([P, ST_W], F16, tag="dstage",
                                     name="dstage")
        else:
            stage = pstage_pool.tile([P, ST_W], F16, tag="pstage",
                                     name="pstage")
        ga = slice(g * ST_W, (g + 1) * ST_W)
        if eng is not None:
            eng.tensor_scalar(
                out=stage[:], in0=a_t[s][0][:, ga],
                scalar1=xb[s][:, 3 * i: 3 * i + 1],
                scalar2=xb[s][:, 9 + i: 10 + i],
                op0=ALU.mult, op1=ALU.add)
            for j in (1, 2):
                eng.scalar_tensor_tensor(
                    out=stage[:], in0=a_t[s][j][:, ga],
                    scalar=xb[s][:, 3 * i + j: 3 * i + j + 1],
                    in1=stage[:], op0=ALU.mult, op1=ALU.add)
        else:
            for seg in range(ST_W // PS_OUT_W):
                pt = ps_out.tile([P, PS_OUT_W], F32, tag="pt", name="pt")
                o0 = g * ST_W + seg * PS_OUT_W
                for j in range(C):
                    nc.tensor.matmul(
                        pt[:], dg[s][i][j][:],
                        a_t[s][j][:, o0:o0 + PS_OUT_W],
                        start=(j == 0), stop=(j == 2))
                nc.scalar.add(
                    out=stage[:, seg * PS_OUT_W:(seg + 1) * PS_OUT_W],
                    in_=pt[:], add=xb[s][:, 9 + i:10 + i])
        nc.sync.dma_start(out=outv[s, i][:, g * ST_W:(g + 1) * ST_W],
                          in_=stage[:])

    def pass2_units(s):
        units = [(g, i) for g in range(F // ST_W) for i in range(C)]
        dve_set = set(units[-N_P2_DVE:]) if (s == 1 and N_P2_DVE) else set()
        return units, dve_set

    # ---------------- global emission order ----------------
    emit_loads(0)
    emit_loads_a(1)
    emit_loads_b(1, 0, 2)
    emit_alloc_pass1(0)
    emit_alloc_pass1(1)
    emit_pe_aad(0)
    for (c, j) in ba_gram_list(0):
        emit_pe_ba_gram(0, c, j)
    emit_dve_products(0)       # drains s0 extracts between product pairs
    emit_pe_aad(1)             # fills the PE window while solve(0) runs
    emit_extracts(0)
    emit_stats(0)
    emit_solve(0)
    emit_loads_b(1, 2, NQ)
    drain_extracts(1, 4)       # s1 AAd/AAX extracts; ready before products
    emit_dve_products(1)       # barrier tile gates this behind solve(0)

    # s1-BA grams interleaved with s0 pass2 units on the PE queue
    units0 = [(g, i) for g in range(F // ST_W) for i in range(C)]
    ba1 = ba_gram_list(1)
    ui = 0
    per = max(1, len(units0) // max(1, len(ba1)))
    for n, (c, j) in enumerate(ba1):
        emit_pe_ba_gram(1, c, j)
        take = per if n < len(ba1) - 1 else len(units0) - ui
        for _ in range(take):
            if ui < len(units0):
                g, i = units0[ui]
                emit_pass2_unit(0, g, i)
                ui += 1

    emit_extracts(1)
    emit_stats(1)
    emit_solve(1)
    units1 = [(g, i) for g in range(F // ST_W) for i in range(C)]
    dve_pos = set()
    if N_P2_DVE:
        step = max(1, len(units1) // N_P2_DVE)
        k = 0
        while len(dve_pos) < N_P2_DVE and k < len(units1):
            dve_pos.add(k)
            k += step
    for idx, (g, i) in enumerate(units1):
        emit_pass2_unit(1, g, i, eng=nc.vector if idx in dve_pos else None)


def build_nc() -> "bass.Bass":
    nc = bacc.Bacc("TRN2", target_bir_lowering=False)
    src = nc.dram_tensor("src", [B_CORE, C, H, W], F32, kind="ExternalInput")
    dst = nc.dram_tensor("dst", [B_CORE, C, H, W], F32, kind="ExternalInput")
    out = nc.dram_tensor("out", [B_CORE, C, H, W], F16, kind="ExternalOutput")
    with tile.TileContext(nc) as tc:
        _colornorm(tc, src[:], dst[:], out[:])
    nc.finalize()
    return nc


_NC = None


def _get_nc():
    global _NC
    if _NC is None:
        _NC = build_nc()
    return _NC


TRACE = False
LAST_RESULT = None  # BassKernelResults of the most recent run (for profiling)


def kernel(src, dst):
    from concourse.bass_utils import run_bass_kernel_spmd

    global LAST_RESULT
    src = np.ascontiguousarray(np.asarray(src, dtype=np.float32))
    dst = np.ascontiguousarray(np.asarray(dst, dtype=np.float32))
    assert src.shape == (NCORES * B_CORE, C, H, W), src.shape
    nc = _get_nc()
    in_maps = [
        {
            "src": np.ascontiguousarray(src[i * B_CORE:(i + 1) * B_CORE]),
            "dst": np.ascontiguousarray(dst[i * B_CORE:(i + 1) * B_CORE]),
        }
        for i in range(NCORES)
    ]
    res = run_bass_kernel_spmd(nc, in_maps, core_ids=list(range(NCORES)),
                               trace=TRACE)
    LAST_RESULT = res
    return np.concatenate(
        [np.asarray(r["out"]).astype(np.float32) for r in res.results], axis=0)


# revision 8
# speedup vs baseline: 1.8000x; 1.0167x over previous
"""ColorNorm Trainium2 kernel, v2: PE-Gram pass1.

Problem: per-sample 3x3 color-matching solve over N=1024*1024 pixels.
  A = src[b] (3,N), B = dst[b] (3,N)
  AAt = Ac@Ac.T + 1e-3 I ; BAt = Bc@Ac.T ; x = BAt@inv(AAt)
  out[b] = x@Ac + Bmean
Sharding: data-parallel over batch (16 samples -> 8 cores x 2 samples).

v2 design (cost-model driven):
  - fp16 data plane (cast in DMA); fp16 DRAM output (host upcasts), which
    halves store traffic on the serial DMA resource.
  - pass1 pair sums run mostly on PE as chunked Gram matmuls: for each
    128-col chunk q, psum[128,128] += W_chunk.T @ X_chunk accumulated over
    64 chunks; the Gram diagonal holds per-col-offset partials, extracted
    by one DVE tensor_tensor_reduce against an identity into acc columns.
    Raw channel sums ride along as 1-col ones matmuls sharing the loaded
    weights; a cross-partition ones-matmul finishes all stats at once.
  - a few pairs run on DVE (TT+TS) and one on GpSimd to balance engines.
  - the 3x3 solve runs on GpSimd so its long serial chain doesn't get
    head-of-line blocked behind the next sample's big DVE products.
  - pass2 on PE with diag(x_ij) stationary weights; ScalarE evicts with
    +d_i bias fused into an fp16 stage stored via SP HWDGE; a couple of
    tail quarters go through a DVE ts+stt path to shorten the tail.
"""

import os
import sys

for _p in ("/opt/trn_rl_repo", "/opt/pypackages"):
    if _p not in sys.path:
        sys.path.append(_p)

from contextlib import ExitStack

import numpy as np

import concourse.bacc as bacc
import concourse.bass as bass
import concourse.tile as tile
from concourse import bass_isa, masks, mybir
from concourse._compat import with_exitstack

# ---- hardcoded problem geometry (per core) ----
B_CORE = 2          # samples per core
C = 3               # channels
H = W = 1024
N = H * W           # 1048576 pixels per channel
P = 128             # SBUF partitions
F = N // P          # 8192 free elems per partition per channel
QW = 2048           # B load quarter width
NQ = F // QW        # 4 quarters
CHW = 128           # Gram chunk width (psum partition dim)
NCH = F // CHW      # 64 chunks
NCORES = 8
RIDGE = 1e-3

F32 = mybir.dt.float32
F16 = mybir.dt.float16
ALU = mybir.AluOpType

# knobs
N_BA_DVE = int(os.environ.get("CN_BA_DVE", "3"))   # BA pairs on DVE
N_BA_POOL = int(os.environ.get("CN_BA_POOL", "0"))  # BA pairs on GpSimd
N_P2_DVE = int(os.environ.get("CN_P2_DVE", "4"))   # s1 pass2 units on DVE
PS_OUT_W = int(os.environ.get("CN_PSW", "512"))    # pass2 psum tile width
ST_W = 2048                                        # store chunk width
GRAM_MAJOR_TAIL = int(os.environ.get("CN_GMT", "1"))
ILV = os.environ.get("CN_ILV", "qu")  # interleave pattern of s1-BA/pass2-s0

_ALL_BA = [(0, 0), (1, 1), (2, 2), (0, 1), (1, 0), (0, 2), (2, 0),
           (1, 2), (2, 1)]
N_BA_DVE_S = [int(x) for x in
              os.environ.get("CN_BA_DVE_S", "2,3").split(",")]
BA_DVE_PAIRS_S = [_ALL_BA[:n] for n in N_BA_DVE_S]
BA_POOL_PAIRS = []
BA_PE_PAIRS_S = [[p for p in _ALL_BA if p not in BA_DVE_PAIRS_S[s]]
                 for s in range(2)]
A_CROSS_PE = [(0, 1)]
A_CROSS = [(0, 2), (1, 2)]
SYM3 = {(0, 1): 0, (0, 2): 1, (1, 2): 2}

# acc column map (per-partition partials; stats-mm reduces partitions)
#   0-8   BA(c,j) at 3c+j ; 9-17 AA row-major (diag 9+4i, cross 9+3i+j for
#   i<j; lower triangle filled in the solve) ; 18-20 rawA ; 21-23 rawB ;
#   24-26 AA-diag second halves (folded into 9+4i by the solve)
ACC_W = 27
# mstat region: cols of gram bank 2: rawA 384-386, rawB 387-389,
# stats-mm out row0 392-412
MST0 = 384
STAT0 = 392


def _rd(ap, dims):
    """Rebuild an AP keeping its partition dim, replacing free dims."""
    return bass.AP(ap.tensor, ap.offset, [ap.ap[0]] + dims)


@with_exitstack
def _colornorm(ctx: ExitStack, tc: "tile.TileContext", src, dst, out):
    nc = tc.nc
    srcv = src.rearrange("b c (p q) w -> b c p (q w)", p=P)  # [2,3,128,8192]
    dstv = dst.rearrange("b c (p q) w -> b c p (q w)", p=P)
    outv = out.rearrange("b c (p q) w -> b c p (q w)", p=P)

    singles = ctx.enter_context(tc.tile_pool(name="singles", bufs=1))
    a_pool = ctx.enter_context(tc.tile_pool(name="a_pool", bufs=6))
    b_pool = ctx.enter_context(tc.tile_pool(name="b_pool", bufs=12))
    scr_pool = ctx.enter_context(tc.tile_pool(name="scr", bufs=1))
    gscr_pool = ctx.enter_context(tc.tile_pool(name="gscr", bufs=1))
    ex_pool = ctx.enter_context(tc.tile_pool(name="exscr", bufs=2))
    ascr_pool = ctx.enter_context(tc.tile_pool(name="ascr", bufs=2))
    acc_pool = ctx.enter_context(tc.tile_pool(name="accs", bufs=2))
    solve_pool = ctx.enter_context(tc.tile_pool(name="solve", bufs=2))
    dg_pool = ctx.enter_context(tc.tile_pool(name="dg", bufs=2))
    stage_pool = ctx.enter_context(tc.tile_pool(name="stage", bufs=3))
    pstage_pool = ctx.enter_context(tc.tile_pool(name="pstage", bufs=2))
    dstage_pool = ctx.enter_context(tc.tile_pool(name="dstage", bufs=2))
    ps_gram = ctx.enter_context(tc.tile_pool(name="ps_gram", bufs=6,
                                             space="PSUM"))
    ps_out = ctx.enter_context(tc.tile_pool(name="ps_out", bufs=2,
                                            space="PSUM"))

    ones16 = singles.tile([P, 1], F16)
    nc.vector.memset(ones16, 1.0)
    ones32 = singles.tile([P, 1], F32)
    nc.vector.memset(ones32, 1.0)
    one1 = singles.tile([1, 1], F32)
    nc.vector.memset(one1, 1.0)
    eye16 = singles.tile([P, P], F16)
    masks.make_identity(nc, eye16[:])
    eye32 = singles.tile([P, P], F32)
    masks.make_identity(nc, eye32[:])

    # per-sample state
    barrier_scr = [None]
    a_t = [None, None]
    b_t = [None, None]
    acc = [None, None]
    stat_t = [None, None]     # [s] -> [P, 512] psum tile holding stats row
    xb = [None, None]
    dg = [None, None]
    pending_ex = [[], []]     # [s] -> [(gram_tile, acc_col), ...]

    def emit_loads_a(s):
        a_t[s] = [a_pool.tile([P, F], F16, tag="ach", name="ach")
                  for _ in range(C)]
        for c in range(C):
            nc.gpsimd.dma_start(out=a_t[s][c][:], in_=srcv[s, c])
        if b_t[s] is None:
            b_t[s] = [[None] * NQ for _ in range(C)]

    def emit_loads_b(s, qq0, qq1, c_major=False):
        order = ([(c, qq) for c in range(C) for qq in range(qq0, qq1)]
                 if c_major else
                 [(c, qq) for qq in range(qq0, qq1) for c in range(C)])
        for c, qq in order:
            t = b_pool.tile([P, QW], F16, tag="bq", name="bq")
            b_t[s][c][qq] = t
            nc.gpsimd.dma_start(
                out=t[:], in_=dstv[s, c][:, qq * QW:(qq + 1) * QW])

    def emit_loads(s):
        emit_loads_a(s)
        emit_loads_b(s, 0, NQ, c_major=True)

    def emit_alloc_pass1(s):
        acc[s] = acc_pool.tile([P, ACC_W], F32, tag="acc", name="acc")
        nc.vector.memset(acc[s], 0.0)

    def a_chunk(s, c, q):
        return a_t[s][c][:, q * CHW:(q + 1) * CHW]

    def b_chunk(s, c, q):
        qq, lc = q // (QW // CHW), q % (QW // CHW)
        return b_t[s][c][qq][:, lc * CHW:(lc + 1) * CHW]

    def emit_gram(s, kind, x, y, raw=None):
        """One Gram = one whole rotating psum bank; PSUM start zeroes the
        full 2KB zero region per partition, so groups can never share a
        bank. Raw channel sums ride along in their own bank, sharing the
        loaded weights; ScalarE copies them out right after the stop."""
        gt = ps_gram.tile([P, 512], F32, tag="g", name="g")
        rt = ps_gram.tile([P, 512], F32, tag="g", name="g") if raw else None
        for q in range(NCH):
            st, sp = q == 0, q == NCH - 1
            lhs = (a_chunk(s, x, q) if kind in ("AAD", "AAX")
                   else b_chunk(s, x, q))
            nc.tensor.matmul(gt[:, 0:128], lhs, a_chunk(s, y, q),
                             start=st, stop=sp)
            if raw:
                nc.tensor.matmul(rt[:, 0:1], lhs, ones16[:],
                                 start=st, stop=sp)
        if raw:
            rkind, ch = raw
            col = (18 if rkind == "A" else 21) + ch
            nc.scalar.activation(out=acc[s][:, col:col + 1], in_=rt[:, 0:1],
                                 func=mybir.ActivationFunctionType.Copy)
        if kind == "BA":
            col = 3 * x + y
        else:
            col = 9 + (4 * x if kind == "AAD" else 3 * x + y)
        pending_ex[s].append((gt, col))

    def emit_pe_aad(s):
        # PE-cross gram + rawA ones-matmuls (engine-free); the AA diagonals
        # run on ScalarE as Square+accum halves (see emit_act_squares)
        for i in range(C):
            rt = ps_gram.tile([P, 512], F32, tag="g", name="g")
            for q in range(NCH):
                nc.tensor.matmul(rt[:, 0:1], a_chunk(s, i, q), ones16[:],
                                 start=q == 0, stop=q == NCH - 1)
            nc.scalar.activation(out=acc[s][:, 18 + i:19 + i], in_=rt[:, 0:1],
                                 func=mybir.ActivationFunctionType.Copy)
        for (i, j) in A_CROSS_PE:
            emit_gram(s, "AAX", i, j)

    def emit_act_squares(s):
        HB = F // 2
        for i in range(C):
            for h in range(2):
                ascr = ascr_pool.tile([P, HB], F16, tag="ascr", name="ascr")
                col = 9 + 4 * i if h == 0 else 24 + i
                nc.scalar.activation(
                    out=ascr[:], in_=a_t[s][i][:, h * HB:(h + 1) * HB],
                    func=mybir.ActivationFunctionType.Square,
                    accum_out=acc[s][:, col:col + 1])

    def ba_gram_list(s):
        return [(c, j) for c in range(C)
                for j in [j2 for (cc, j2) in BA_PE_PAIRS_S[s] if cc == c]]

    def emit_pe_ba_gram(s, c, j):
        js = [j2 for (cc, j2) in BA_PE_PAIRS_S[s] if cc == c]
        raw = ("B", c) if j == js[-1] else None
        emit_gram(s, "BA", c, j, raw=raw)

    def drain_extracts(s, n=None):
        take = pending_ex[s] if n is None else pending_ex[s][:n]
        pending_ex[s] = [] if n is None else pending_ex[s][n:]
        for gt, col in take:
            # tensor_tensor_reduce crashes the device at runtime, so the
            # diagonal extract is a TT eye-mask + TS accumulate instead
            ex = ex_pool.tile([P, P], F32, tag="ex", name="ex")
            nc.vector.tensor_mul(out=ex[:], in0=gt[:, 0:128], in1=eye32[:])
            nc.vector.tensor_scalar(
                out=ex[:], in0=ex[:], scalar1=1.0, scalar2=0.0,
                op0=ALU.mult, op1=ALU.add,
                accum_out=acc[s][:, col:col + 1])

    def emit_dve_products(s):
        # AA cross pairs: full-width TT + TS accum
        for k, (i, j) in enumerate(A_CROSS):
            drain_extracts(s, 3)
            if s == 1 and barrier_scr[0] is not None:
                scr = barrier_scr[0]
                barrier_scr[0] = None
            else:
                scr = scr_pool.tile([P, F], F16, tag="scr", name="scr")
            nc.vector.tensor_mul(out=scr[:], in0=a_t[s][i][:],
                                 in1=a_t[s][j][:])
            nc.vector.tensor_scalar(
                out=scr[:], in0=scr[:], scalar1=1.0, scalar2=0.0,
                op0=ALU.mult, op1=ALU.add,
                accum_out=acc[s][:, 9 + 3 * i + j:10 + 3 * i + j])
        # BA pairs on DVE / GpSimd: per-quarter TT, one DVE TS accum
        for (c, j) in BA_DVE_PAIRS_S[s] + BA_POOL_PAIRS:
            drain_extracts(s, 2)
            on_pool = (c, j) in BA_POOL_PAIRS
            pool_p = gscr_pool if on_pool else scr_pool
            eng = nc.gpsimd if on_pool else nc.vector
            scr = pool_p.tile([P, F], F16, tag="pscr" if on_pool else "scr",
                              name="pscr" if on_pool else "scr")
            for qq in range(NQ):
                eng.tensor_mul(
                    out=scr[:, qq * QW:(qq + 1) * QW], in0=b_t[s][c][qq][:],
                    in1=a_t[s][j][:, qq * QW:(qq + 1) * QW])
            nc.vector.tensor_scalar(
                out=scr[:], in0=scr[:], scalar1=1.0, scalar2=0.0,
                op0=ALU.mult, op1=ALU.add,
                accum_out=acc[s][:, 3 * c + j:3 * c + j + 1])

    def emit_extracts(s):
        drain_extracts(s)

    def emit_stats(s):
        stat_t[s] = ps_gram.tile([P, 512], F32, tag="g", name="g")
        nc.tensor.matmul(stat_t[s][0:1, 0:ACC_W], ones32[:],
                         acc[s][:], start=True, stop=True)

    def emit_solve(s):
        # solve on DVE. The serial chain must not share the engine with big
        # ops that are concurrently ready (they bypass stalled ops via the
        # exec window and every hop then waits behind a multi-us op), so
        # the next sample's DVE products are gated on a barrier tile that
        # the end of this solve writes (see emit_dve_products).
        v = nc.vector
        stats = solve_pool.tile([1, ACC_W], F32, tag="stats", name="stats")
        v.tensor_copy(out=stats[:], in_=stat_t[s][0:1, 0:ACC_W])

        # fold AA-diag second halves: cols (9,13,17) += (24,25,26)
        v.tensor_add(out=_rd(stats[0:1, 9:10], [[4, 3]]),
                     in0=_rd(stats[0:1, 9:10], [[4, 3]]),
                     in1=stats[0:1, 24:27])
        # fill AA lower triangle from upper: cols (12,15)<-(10,11); 16<-14
        v.tensor_copy(out=_rd(stats[0:1, 12:13], [[3, 2]]),
                      in_=stats[0:1, 10:12])
        v.tensor_copy(out=stats[0:1, 16:17], in_=stats[0:1, 14:15])

        # means: one op over rawA|rawB
        Am6 = solve_pool.tile([1, 6], F32, tag="Am6", name="Am6")
        v.tensor_scalar_mul(out=Am6[:], in0=stats[0:1, 18:24],
                            scalar1=1.0 / N)
        Am = Am6[0:1, 0:3]
        Bm = Am6[0:1, 3:6]

        outer = solve_pool.tile([1, 9], F32, tag="outer", name="outer")
        o3x3 = outer[0:1, :].rearrange("p (i j) -> p i j", j=3)
        v.tensor_mul(out=o3x3, in0=_rd(Am[0:1, 0:1], [[1, 3], [0, 3]]),
                     in1=_rd(Am[0:1, 0:1], [[0, 3], [1, 3]]))
        AAc = solve_pool.tile([1, 9], F32, tag="AAc", name="AAc")
        v.scalar_tensor_tensor(out=AAc[:], in0=outer[:], scalar=-float(N),
                               in1=stats[0:1, 9:18], op0=ALU.mult,
                               op1=ALU.add)
        dg_ap = _rd(AAc[0:1, 0:1], [[4, 3]])
        v.tensor_scalar_add(out=dg_ap, in0=dg_ap, scalar1=RIDGE)
        v.tensor_mul(out=o3x3, in0=_rd(Bm[0:1, 0:1], [[1, 3], [0, 3]]),
                     in1=_rd(Am[0:1, 0:1], [[0, 3], [1, 3]]))
        BAc = solve_pool.tile([1, 9], F32, tag="BAc", name="BAc")
        v.scalar_tensor_tensor(out=BAc[:], in0=outer[:], scalar=-float(N),
                               in1=stats[0:1, 0:9], op0=ALU.mult,
                               op1=ALU.add)

        # M2 = 2x2 tiling of AAc in one 4-dim strided copy
        M2 = solve_pool.tile([1, 36], F32, tag="M2", name="M2")
        v.tensor_copy(
            out=_rd(M2[0:1, 0:1], [[18, 2], [3, 2], [6, 3], [1, 3]]),
            in_=_rd(AAc[0:1, 0:1], [[0, 2], [0, 2], [3, 3], [1, 3]]))
        t1 = solve_pool.tile([1, 9], F32, tag="t1", name="t1")
        t2 = solve_pool.tile([1, 9], F32, tag="t2", name="t2")
        v.tensor_mul(out=t1[0:1, :].rearrange("p (i j) -> p i j", j=3),
                     in0=_rd(M2[0:1, 7:8], [[6, 3], [1, 3]]),
                     in1=_rd(M2[0:1, 14:15], [[6, 3], [1, 3]]))
        v.tensor_mul(out=t2[0:1, :].rearrange("p (i j) -> p i j", j=3),
                     in0=_rd(M2[0:1, 8:9], [[6, 3], [1, 3]]),
                     in1=_rd(M2[0:1, 13:14], [[6, 3], [1, 3]]))
        cof = solve_pool.tile([1, 9], F32, tag="cof", name="cof")
        v.tensor_sub(out=cof[:], in0=t1[:], in1=t2[:])

        det = solve_pool.tile([1, 1], F32, tag="det", name="det")
        dscr = solve_pool.tile([1, 3], F32, tag="dscr", name="dscr")
        v.scalar_tensor_tensor(
            out=dscr[:], in0=AAc[0:1, 0:3], scalar=1.0, in1=cof[0:1, 0:3],
            op0=ALU.mult, op1=ALU.mult, accum_out=det[:])
        rdet = solve_pool.tile([1, 1], F32, tag="rdet", name="rdet")
        v.reciprocal(out=rdet[:], in_=det[:])

        inv9 = solve_pool.tile([1, 9], F32, tag="inv9", name="inv9")
        v.tensor_scalar_mul(
            out=inv9[0:1, :].rearrange("p (i j) -> p i j", j=3),
            in0=_rd(cof[0:1, 0:1], [[1, 3], [3, 3]]),  # cof^T
            scalar1=rdet[:])

        tmp27 = solve_pool.tile([1, 27], F32, tag="tmp27", name="tmp27")
        v.tensor_mul(
            out=tmp27[0:1, :].rearrange("p (i k j) -> p i k j", k=3, j=3),
            in0=_rd(BAc[0:1, 0:1], [[3, 3], [0, 3], [1, 3]]),
            in1=_rd(inv9[0:1, 0:1], [[0, 3], [1, 3], [3, 3]]))
        # x9 into sol[0:9]; d = Bm - x@Am into sol[9:12]
        sol = solve_pool.tile([1, 12], F32, tag="sol", name="sol")
        x9 = sol[0:1, 0:9]
        v.tensor_add(out=x9, in0=_rd(tmp27[0:1, 0:1], [[3, 9]]),
                     in1=_rd(tmp27[0:1, 1:2], [[3, 9]]))
        v.tensor_add(out=x9, in0=x9,
                     in1=_rd(tmp27[0:1, 2:3], [[3, 9]]))

        tmp9 = solve_pool.tile([1, 9], F32, tag="tmp9", name="tmp9")
        v.tensor_mul(
            out=tmp9[0:1, :].rearrange("p (i j) -> p i j", j=3),
            in0=x9.rearrange("p (i j) -> p i j", j=3),
            in1=_rd(Am[0:1, 0:1], [[0, 3], [1, 3]]))
        xAm = solve_pool.tile([1, 3], F32, tag="xAm", name="xAm")
        v.tensor_add(out=xAm[:], in0=_rd(tmp9[0:1, 0:1], [[3, 3]]),
                     in1=_rd(tmp9[0:1, 1:2], [[3, 3]]))
        v.tensor_add(out=xAm[:], in0=xAm[:],
                     in1=_rd(tmp9[0:1, 2:3], [[3, 3]]))
        v.tensor_sub(out=sol[0:1, 9:12], in0=Bm[:], in1=xAm[:])

        xb[s] = solve_pool.tile([P, 12], F32, tag="xb", name="xb")
        nc.gpsimd.partition_broadcast(xb[s][:], sol[0:1, 0:12])

        # all 9 diag(x_ij) weight tiles in one broadcast-AP tensor_mul
        xb16 = solve_pool.tile([P, 12], F16, tag="xb16", name="xb16")
        v.tensor_copy(out=xb16[:], in_=xb[s][:])
        dgall = dg_pool.tile([P, 9 * P], F16, tag="dgall", name="dgall")
        v.tensor_mul(out=_rd(dgall[:, 0:1], [[128, 9], [1, 128]]),
                     in0=_rd(eye16[:, 0:1], [[0, 9], [1, 128]]),
                     in1=_rd(xb16[:, 0:1], [[1, 9], [0, 128]]))
        dg[s] = [[dgall[:, (3 * i + j) * P:(3 * i + j + 1) * P]
                  for j in range(C)] for i in range(C)]
        if s == 0:
            # barrier tile: sample 1's first DVE product writes this tile,
            # so it cannot start (or bypass-run) before the solve finishes
            bar = scr_pool.tile([P, F], F16, tag="scr", name="scr")
            v.tensor_copy(out=bar[0:1, 0:1], in_=dgall[0:1, 0:1])
            barrier_scr[0] = bar

    def emit_pass2_unit(s, g, i, eng=None):
        if eng is None:
            stage = stage_pool.tile([P, ST_W], F16, tag="stage", name="stage")
        elif eng is nc.vector:
            stage = dstage_pool.tile([P, ST_W], F16, tag="dstage",
                                     name="dstage")
        else:
            stage = pstage_pool.tile([P, ST_W], F16, tag="pstage",
                                     name="pstage")
        ga = slice(g * ST_W, (g + 1) * ST_W)
        if eng is not None:
            eng.tensor_scalar(
                out=stage[:], in0=a_t[s][0][:, ga],
                scalar1=xb[s][:, 3 * i: 3 * i + 1],
                scalar2=xb[s][:, 9 + i: 10 + i],
                op0=ALU.mult, op1=ALU.add)
            for j in (1, 2):
                eng.scalar_tensor_tensor(
                    out=stage[:], in0=a_t[s][j][:, ga],
                    scalar=xb[s][:, 3 * i + j: 3 * i + j + 1],
                    in1=stage[:], op0=ALU.mult, op1=ALU.add)
        else:
            for seg in range(ST_W // PS_OUT_W):
                pt = ps_out.tile([P, PS_OUT_W], F32, tag="pt", name="pt")
                o0 = g * ST_W + seg * PS_OUT_W
                for j in range(C):
                    nc.tensor.matmul(
                        pt[:], dg[s][i][j][:],
                        a_t[s][j][:, o0:o0 + PS_OUT_W],
                        start=(j == 0), stop=(j == 2))
                nc.scalar.add(
                    out=stage[:, seg * PS_OUT_W:(seg + 1) * PS_OUT_W],
                    in_=pt[:], add=xb[s][:, 9 + i:10 + i])
        nc.sync.dma_start(out=outv[s, i][:, g * ST_W:(g + 1) * ST_W],
                          in_=stage[:])

    def pass2_units(s):
        units = [(g, i) for g in range(F // ST_W) for i in range(C)]
        dve_set = set(units[-N_P2_DVE:]) if (s == 1 and N_P2_DVE) else set()
        return units, dve_set

    # ---------------- global emission order ----------------
    emit_loads(0)
    emit_loads_a(1)
    emit_loads_b(1, 0, 2)
    emit_alloc_pass1(0)
    emit_alloc_pass1(1)
    emit_act_squares(0)
    emit_pe_aad(0)
    for (c, j) in ba_gram_list(0):
        emit_pe_ba_gram(0, c, j)
    emit_dve_products(0)       # drains s0 extracts between product pairs
    emit_pe_aad(1)             # fills the PE window while solve(0) runs
    emit_act_squares(1)
    emit_extracts(0)
    emit_stats(0)
    emit_solve(0)
    emit_loads_b(1, 2, NQ)
    drain_extracts(1, 1)       # s1 AAX extract; ready before products
    emit_dve_products(1)       # barrier tile gates this behind solve(0)

    # s1-BA grams interleaved with s0 pass2 units on the PE queue
    units0 = [(g, i) for g in range(F // ST_W) for i in range(C)]
    ba1 = ba_gram_list(1)
    ui = 0
    per = max(1, len(units0) // max(1, len(ba1)))
    for n, (c, j) in enumerate(ba1):
        emit_pe_ba_gram(1, c, j)
        take = per if n < len(ba1) - 1 else len(units0) - ui
        for _ in range(take):
            if ui < len(units0):
                g, i = units0[ui]
                emit_pass2_unit(0, g, i)
                ui += 1

    emit_extracts(1)
    emit_stats(1)
    emit_solve(1)
    units1 = [(g, i) for g in range(F // ST_W) for i in range(C)]
    dve_pos = set()
    if N_P2_DVE:
        step = max(1, len(units1) // N_P2_DVE)
        k = 0
        while len(dve_pos) < N_P2_DVE and k < len(units1):
            dve_pos.add(k)
            k += step
    for idx, (g, i) in enumerate(units1):
        emit_pass2_unit(1, g, i, eng=nc.vector if idx in dve_pos else None)


def build_nc() -> "bass.Bass":
    nc = bacc.Bacc("TRN2", target_bir_lowering=False)
    src = nc.dram_tensor("src", [B_CORE, C, H, W], F32, kind="ExternalInput")
    dst = nc.dram_tensor("dst", [B_CORE, C, H, W], F32, kind="ExternalInput")
    out = nc.dram_tensor("out", [B_CORE, C, H, W], F16, kind="ExternalOutput")
    with tile.TileContext(nc) as tc:
        _colornorm(tc, src[:], dst[:], out[:])
    nc.finalize()
    return nc


_NC = None


def _get_nc():
    global _NC
    if _NC is None:
        _NC = build_nc()
    return _NC


TRACE = False
LAST_RESULT = None  # BassKernelResults of the most recent run (for profiling)


def kernel(src, dst):
    from concourse.bass_utils import run_bass_kernel_spmd

    global LAST_RESULT
    src = np.ascontiguousarray(np.asarray(src, dtype=np.float32))
    dst = np.ascontiguousarray(np.asarray(dst, dtype=np.float32))
    assert src.shape == (NCORES * B_CORE, C, H, W), src.shape
    nc = _get_nc()
    in_maps = [
        {
            "src": np.ascontiguousarray(src[i * B_CORE:(i + 1) * B_CORE]),
            "dst": np.ascontiguousarray(dst[i * B_CORE:(i + 1) * B_CORE]),
        }
        for i in range(NCORES)
    ]
    res = run_bass_kernel_spmd(nc, in_maps, core_ids=list(range(NCORES)),
                               trace=TRACE)
    LAST_RESULT = res
    return np.concatenate(
        [np.asarray(r["out"]).astype(np.float32) for r in res.results], axis=0)


# revision 10
# speedup vs baseline: 1.8105x; 1.0059x over previous
"""ColorNorm Trainium2 kernel, v2: PE-Gram pass1.

Problem: per-sample 3x3 color-matching solve over N=1024*1024 pixels.
  A = src[b] (3,N), B = dst[b] (3,N)
  AAt = Ac@Ac.T + 1e-3 I ; BAt = Bc@Ac.T ; x = BAt@inv(AAt)
  out[b] = x@Ac + Bmean
Sharding: data-parallel over batch (16 samples -> 8 cores x 2 samples).

v2 design (cost-model driven):
  - fp16 data plane (cast in DMA); fp16 DRAM output (host upcasts), which
    halves store traffic on the serial DMA resource.
  - pass1 pair sums run mostly on PE as chunked Gram matmuls: for each
    128-col chunk q, psum[128,128] += W_chunk.T @ X_chunk accumulated over
    64 chunks; the Gram diagonal holds per-col-offset partials, extracted
    by one DVE tensor_tensor_reduce against an identity into acc columns.
    Raw channel sums ride along as 1-col ones matmuls sharing the loaded
    weights; a cross-partition ones-matmul finishes all stats at once.
  - a few pairs run on DVE (TT+TS) and one on GpSimd to balance engines.
  - the 3x3 solve runs on GpSimd so its long serial chain doesn't get
    head-of-line blocked behind the next sample's big DVE products.
  - pass2 on PE with diag(x_ij) stationary weights; ScalarE evicts with
    +d_i bias fused into an fp16 stage stored via SP HWDGE; a couple of
    tail quarters go through a DVE ts+stt path to shorten the tail.
"""

import os
import sys

for _p in ("/opt/trn_rl_repo", "/opt/pypackages"):
    if _p not in sys.path:
        sys.path.append(_p)

from contextlib import ExitStack

import numpy as np

import concourse.bacc as bacc
import concourse.bass as bass
import concourse.tile as tile
from concourse import bass_isa, masks, mybir
from concourse._compat import with_exitstack

# ---- hardcoded problem geometry (per core) ----
B_CORE = 2          # samples per core
C = 3               # channels
H = W = 1024
N = H * W           # 1048576 pixels per channel
P = 128             # SBUF partitions
F = N // P          # 8192 free elems per partition per channel
QW = 2048           # B load quarter width
NQ = F // QW        # 4 quarters
CHW = 128           # Gram chunk width (psum partition dim)
NCH = F // CHW      # 64 chunks
NCORES = 8
RIDGE = 1e-3

F32 = mybir.dt.float32
F16 = mybir.dt.float16
ALU = mybir.AluOpType

# knobs
N_BA_DVE = int(os.environ.get("CN_BA_DVE", "3"))   # BA pairs on DVE
N_BA_POOL = int(os.environ.get("CN_BA_POOL", "0"))  # BA pairs on GpSimd
N_P2_DVE = int(os.environ.get("CN_P2_DVE", "5"))   # s1 pass2 units on DVE
PS_OUT_W = int(os.environ.get("CN_PSW", "512"))    # pass2 psum tile width
ST_W = 2048                                        # store chunk width
GRAM_MAJOR_TAIL = int(os.environ.get("CN_GMT", "1"))
ILV = os.environ.get("CN_ILV", "qu")  # interleave pattern of s1-BA/pass2-s0

_ALL_BA = [(0, 0), (1, 1), (2, 2), (0, 1), (1, 0), (0, 2), (2, 0),
           (1, 2), (2, 1)]
N_BA_DVE_S = [int(x) for x in
              os.environ.get("CN_BA_DVE_S", "3,3").split(",")]
BA_DVE_PAIRS_S = [_ALL_BA[:n] for n in N_BA_DVE_S]
BA_POOL_PAIRS = []
BA_PE_PAIRS_S = [[p for p in _ALL_BA if p not in BA_DVE_PAIRS_S[s]]
                 for s in range(2)]
A_CROSS_PE = [(0, 1)]
A_CROSS = [(0, 2), (1, 2)]
SYM3 = {(0, 1): 0, (0, 2): 1, (1, 2): 2}

# acc column map (per-partition partials; stats-mm reduces partitions)
#   0-8   BA(c,j) at 3c+j ; 9-17 AA row-major (diag 9+4i, cross 9+3i+j for
#   i<j; lower triangle filled in the solve) ; 18-20 rawA ; 21-23 rawB
ACC_W = 24
# mstat region: cols of gram bank 2: rawA 384-386, rawB 387-389,
# stats-mm out row0 392-412
MST0 = 384
STAT0 = 392


def _rd(ap, dims):
    """Rebuild an AP keeping its partition dim, replacing free dims."""
    return bass.AP(ap.tensor, ap.offset, [ap.ap[0]] + dims)


@with_exitstack
def _colornorm(ctx: ExitStack, tc: "tile.TileContext", src, dst, out):
    nc = tc.nc
    srcv = src.rearrange("b c (p q) w -> b c p (q w)", p=P)  # [2,3,128,8192]
    dstv = dst.rearrange("b c (p q) w -> b c p (q w)", p=P)
    outv = out.rearrange("b c (p q) w -> b c p (q w)", p=P)

    singles = ctx.enter_context(tc.tile_pool(name="singles", bufs=1))
    a_pool = ctx.enter_context(tc.tile_pool(name="a_pool", bufs=6))
    b_pool = ctx.enter_context(tc.tile_pool(name="b_pool", bufs=12))
    scr_pool = ctx.enter_context(tc.tile_pool(name="scr", bufs=1))
    gscr_pool = ctx.enter_context(tc.tile_pool(name="gscr", bufs=1))
    ex_pool = ctx.enter_context(tc.tile_pool(name="exscr", bufs=2))
    ascr_pool = ctx.enter_context(tc.tile_pool(name="ascr", bufs=1))
    acc_pool = ctx.enter_context(tc.tile_pool(name="accs", bufs=2))
    solve_pool = ctx.enter_context(tc.tile_pool(name="solve", bufs=2))
    dg_pool = ctx.enter_context(tc.tile_pool(name="dg", bufs=2))
    stage_pool = ctx.enter_context(tc.tile_pool(name="stage", bufs=3))
    pstage_pool = ctx.enter_context(tc.tile_pool(name="pstage", bufs=2))
    dstage_pool = ctx.enter_context(tc.tile_pool(name="dstage", bufs=2))
    ps_gram = ctx.enter_context(tc.tile_pool(name="ps_gram", bufs=6,
                                             space="PSUM"))
    ps_out = ctx.enter_context(tc.tile_pool(name="ps_out", bufs=2,
                                            space="PSUM"))

    ones16 = singles.tile([P, 1], F16)
    nc.vector.memset(ones16, 1.0)
    ones32 = singles.tile([P, 1], F32)
    nc.vector.memset(ones32, 1.0)
    one1 = singles.tile([1, 1], F32)
    nc.vector.memset(one1, 1.0)
    eye16 = singles.tile([P, P], F16)
    masks.make_identity(nc, eye16[:])
    eye32 = singles.tile([P, P], F32)
    masks.make_identity(nc, eye32[:])

    # per-sample state
    barrier_scr = [None]
    a_t = [None, None]
    b_t = [None, None]
    acc = [None, None]
    stat_t = [None, None]     # [s] -> [P, 512] psum tile holding stats row
    xb = [None, None]
    dg = [None, None]
    pending_ex = [[], []]     # [s] -> [(gram_tile, acc_col), ...]

    def emit_loads_a(s):
        a_t[s] = [a_pool.tile([P, F], F16, tag="ach", name="ach")
                  for _ in range(C)]
        for c in range(C):
            nc.gpsimd.dma_start(out=a_t[s][c][:], in_=srcv[s, c])
        if b_t[s] is None:
            b_t[s] = [[None] * NQ for _ in range(C)]

    def emit_loads_b(s, qq0, qq1, c_major=False):
        order = ([(c, qq) for c in range(C) for qq in range(qq0, qq1)]
                 if c_major else
                 [(c, qq) for qq in range(qq0, qq1) for c in range(C)])
        for c, qq in order:
            t = b_pool.tile([P, QW], F16, tag="bq", name="bq")
            b_t[s][c][qq] = t
            nc.gpsimd.dma_start(
                out=t[:], in_=dstv[s, c][:, qq * QW:(qq + 1) * QW])

    def emit_loads(s):
        if int(os.environ.get("CN_LOADMIX", "0")):
            a_t[s] = [a_pool.tile([P, F], F16, tag="ach", name="ach")
                      for _ in range(C)]
            if b_t[s] is None:
                b_t[s] = [[None] * NQ for _ in range(C)]
            def _a(c):
                nc.gpsimd.dma_start(out=a_t[s][c][:], in_=srcv[s, c])
            def _b(c):
                for qq in range(NQ):
                    t = b_pool.tile([P, QW], F16, tag="bq", name="bq")
                    b_t[s][c][qq] = t
                    nc.gpsimd.dma_start(
                        out=t[:], in_=dstv[s, c][:, qq * QW:(qq + 1) * QW])
            _a(0); _a(1); _b(0); _a(2); _b(1); _b(2)
        else:
            emit_loads_a(s)
            emit_loads_b(s, 0, NQ, c_major=True)

    def emit_alloc_pass1(s):
        acc[s] = acc_pool.tile([P, ACC_W], F32, tag="acc", name="acc")
        nc.vector.memset(acc[s], 0.0)

    def a_chunk(s, c, q):
        return a_t[s][c][:, q * CHW:(q + 1) * CHW]

    def b_chunk(s, c, q):
        qq, lc = q // (QW // CHW), q % (QW // CHW)
        return b_t[s][c][qq][:, lc * CHW:(lc + 1) * CHW]

    def emit_gram(s, kind, x, y, raw=None):
        """One Gram = one whole rotating psum bank; PSUM start zeroes the
        full 2KB zero region per partition, so groups can never share a
        bank. Raw channel sums ride along in their own bank, sharing the
        loaded weights; ScalarE copies them out right after the stop."""
        gt = ps_gram.tile([P, 512], F32, tag="g", name="g")
        rt = ps_gram.tile([P, 512], F32, tag="g", name="g") if raw else None
        for q in range(NCH):
            st, sp = q == 0, q == NCH - 1
            lhs = (a_chunk(s, x, q) if kind in ("AAD", "AAX")
                   else b_chunk(s, x, q))
            nc.tensor.matmul(gt[:, 0:128], lhs, a_chunk(s, y, q),
                             start=st, stop=sp)
            if raw:
                nc.tensor.matmul(rt[:, 0:1], lhs, ones16[:],
                                 start=st, stop=sp)
        if raw:
            rkind, ch = raw
            col = (18 if rkind == "A" else 21) + ch
            nc.scalar.activation(out=acc[s][:, col:col + 1], in_=rt[:, 0:1],
                                 func=mybir.ActivationFunctionType.Copy)
        if kind == "BA":
            col = 3 * x + y
        else:
            col = 9 + (4 * x if kind == "AAD" else 3 * x + y)
        pending_ex[s].append((gt, col))

    def emit_pe_aad(s):
        # PE-cross gram + rawA ones-matmuls (engine-free); the AA diagonals
        # run on ScalarE as Square+accum halves (see emit_act_squares)
        for i in range(C):
            rt = ps_gram.tile([P, 512], F32, tag="g", name="g")
            for q in range(NCH):
                nc.tensor.matmul(rt[:, 0:1], a_chunk(s, i, q), ones16[:],
                                 start=q == 0, stop=q == NCH - 1)
            nc.scalar.activation(out=acc[s][:, 18 + i:19 + i], in_=rt[:, 0:1],
                                 func=mybir.ActivationFunctionType.Copy)
        for (i, j) in A_CROSS_PE:
            emit_gram(s, "AAX", i, j)

    def emit_act_squares(s):
        for i in range(C):
            ascr = ascr_pool.tile([P, F], F16, tag="ascr", name="ascr")
            nc.scalar.activation(
                out=ascr[:], in_=a_t[s][i][:],
                func=mybir.ActivationFunctionType.Square,
                accum_out=acc[s][:, 9 + 4 * i:10 + 4 * i])

    def ba_gram_list(s):
        return [(c, j) for c in range(C)
                for j in [j2 for (cc, j2) in BA_PE_PAIRS_S[s] if cc == c]]

    def emit_pe_ba_gram(s, c, j):
        js = [j2 for (cc, j2) in BA_PE_PAIRS_S[s] if cc == c]
        raw = ("B", c) if j == js[-1] else None
        emit_gram(s, "BA", c, j, raw=raw)

    def drain_extracts(s, n=None):
        take = pending_ex[s] if n is None else pending_ex[s][:n]
        pending_ex[s] = [] if n is None else pending_ex[s][n:]
        for gt, col in take:
            # tensor_tensor_reduce crashes the device at runtime, so the
            # diagonal extract is a TT eye-mask + TS accumulate instead
            ex = ex_pool.tile([P, P], F32, tag="ex", name="ex")
            nc.vector.tensor_mul(out=ex[:], in0=gt[:, 0:128], in1=eye32[:])
            nc.vector.tensor_scalar(
                out=ex[:], in0=ex[:], scalar1=1.0, scalar2=0.0,
                op0=ALU.mult, op1=ALU.add,
                accum_out=acc[s][:, col:col + 1])

    def emit_dve_products(s):
        # AA cross pairs: full-width TT + TS accum
        for k, (i, j) in enumerate(A_CROSS):
            drain_extracts(s, 3)
            if s == 1 and barrier_scr[0] is not None:
                scr = barrier_scr[0]
                barrier_scr[0] = None
            else:
                scr = scr_pool.tile([P, F], F16, tag="scr", name="scr")
            nc.vector.tensor_mul(out=scr[:], in0=a_t[s][i][:],
                                 in1=a_t[s][j][:])
            nc.vector.tensor_scalar(
                out=scr[:], in0=scr[:], scalar1=1.0, scalar2=0.0,
                op0=ALU.mult, op1=ALU.add,
                accum_out=acc[s][:, 9 + 3 * i + j:10 + 3 * i + j])
        # BA pairs on DVE / GpSimd: per-quarter TT, one DVE TS accum
        for (c, j) in BA_DVE_PAIRS_S[s] + BA_POOL_PAIRS:
            drain_extracts(s, 2)
            on_pool = (c, j) in BA_POOL_PAIRS
            pool_p = gscr_pool if on_pool else scr_pool
            eng = nc.gpsimd if on_pool else nc.vector
            scr = pool_p.tile([P, F], F16, tag="pscr" if on_pool else "scr",
                              name="pscr" if on_pool else "scr")
            for qq in range(NQ):
                eng.tensor_mul(
                    out=scr[:, qq * QW:(qq + 1) * QW], in0=b_t[s][c][qq][:],
                    in1=a_t[s][j][:, qq * QW:(qq + 1) * QW])
            nc.vector.tensor_scalar(
                out=scr[:], in0=scr[:], scalar1=1.0, scalar2=0.0,
                op0=ALU.mult, op1=ALU.add,
                accum_out=acc[s][:, 3 * c + j:3 * c + j + 1])

    def emit_extracts(s):
        drain_extracts(s)

    def emit_stats(s):
        stat_t[s] = ps_gram.tile([P, 512], F32, tag="g", name="g")
        nc.tensor.matmul(stat_t[s][0:1, 0:ACC_W], ones32[:],
                         acc[s][:], start=True, stop=True)

    def emit_solve(s):
        # solve on DVE. The serial chain must not share the engine with big
        # ops that are concurrently ready (they bypass stalled ops via the
        # exec window and every hop then waits behind a multi-us op), so
        # the next sample's DVE products are gated on a barrier tile that
        # the end of this solve writes (see emit_dve_products).
        v = nc.vector
        stats = solve_pool.tile([1, ACC_W], F32, tag="stats", name="stats")
        v.tensor_copy(out=stats[:], in_=stat_t[s][0:1, 0:ACC_W])

        # fill AA lower triangle from upper: cols (12,15)<-(10,11); 16<-14
        v.tensor_copy(out=_rd(stats[0:1, 12:13], [[3, 2]]),
                      in_=stats[0:1, 10:12])
        v.tensor_copy(out=stats[0:1, 16:17], in_=stats[0:1, 14:15])

        # means: one op over rawA|rawB
        Am6 = solve_pool.tile([1, 6], F32, tag="Am6", name="Am6")
        v.tensor_scalar_mul(out=Am6[:], in0=stats[0:1, 18:24],
                            scalar1=1.0 / N)
        Am = Am6[0:1, 0:3]
        Bm = Am6[0:1, 3:6]

        # one TT computes both outer products (rows: Bm*Am^T then Am*Am^T,
        # matching the BA|AA layout of stats cols 0..17), one STT centers
        outer = solve_pool.tile([1, 18], F32, tag="outer", name="outer")
        v.tensor_mul(
            out=_rd(outer[0:1, 0:1], [[9, 2], [3, 3], [1, 3]]),
            in0=_rd(Am6[0:1, 3:4], [[-3, 2], [1, 3], [0, 3]]),
            in1=_rd(Am6[0:1, 0:1], [[0, 2], [0, 3], [1, 3]]))
        CC = solve_pool.tile([1, 18], F32, tag="CC", name="CC")
        v.scalar_tensor_tensor(out=CC[:], in0=outer[:], scalar=-float(N),
                               in1=stats[0:1, 0:18], op0=ALU.mult,
                               op1=ALU.add)
        BAc = CC[0:1, 0:9]
        AAc = CC[0:1, 9:18]
        dg_ap = _rd(CC[0:1, 9:10], [[4, 3]])
        v.tensor_scalar_add(out=dg_ap, in0=dg_ap, scalar1=RIDGE)

        # M2 = 2x2 tiling of AAc in one 4-dim strided copy
        M2 = solve_pool.tile([1, 36], F32, tag="M2", name="M2")
        v.tensor_copy(
            out=_rd(M2[0:1, 0:1], [[18, 2], [3, 2], [6, 3], [1, 3]]),
            in_=_rd(CC[0:1, 9:10], [[0, 2], [0, 2], [3, 3], [1, 3]]))
        t1 = solve_pool.tile([1, 9], F32, tag="t1", name="t1")
        t2 = solve_pool.tile([1, 9], F32, tag="t2", name="t2")
        v.tensor_mul(out=t1[0:1, :].rearrange("p (i j) -> p i j", j=3),
                     in0=_rd(M2[0:1, 7:8], [[6, 3], [1, 3]]),
                     in1=_rd(M2[0:1, 14:15], [[6, 3], [1, 3]]))
        v.tensor_mul(out=t2[0:1, :].rearrange("p (i j) -> p i j", j=3),
                     in0=_rd(M2[0:1, 8:9], [[6, 3], [1, 3]]),
                     in1=_rd(M2[0:1, 13:14], [[6, 3], [1, 3]]))
        cof = solve_pool.tile([1, 9], F32, tag="cof", name="cof")
        v.tensor_sub(out=cof[:], in0=t1[:], in1=t2[:])

        det = solve_pool.tile([1, 1], F32, tag="det", name="det")
        dscr = solve_pool.tile([1, 3], F32, tag="dscr", name="dscr")
        v.scalar_tensor_tensor(
            out=dscr[:], in0=CC[0:1, 9:12], scalar=1.0, in1=cof[0:1, 0:3],
            op0=ALU.mult, op1=ALU.mult, accum_out=det[:])
        rdet = solve_pool.tile([1, 1], F32, tag="rdet", name="rdet")
        v.reciprocal(out=rdet[:], in_=det[:])

        inv9 = solve_pool.tile([1, 9], F32, tag="inv9", name="inv9")
        v.tensor_scalar_mul(
            out=inv9[0:1, :].rearrange("p (i j) -> p i j", j=3),
            in0=_rd(cof[0:1, 0:1], [[1, 3], [3, 3]]),  # cof^T
            scalar1=rdet[:])

        tmp27 = solve_pool.tile([1, 27], F32, tag="tmp27", name="tmp27")
        v.tensor_mul(
            out=tmp27[0:1, :].rearrange("p (i k j) -> p i k j", k=3, j=3),
            in0=_rd(CC[0:1, 0:1], [[3, 3], [0, 3], [1, 3]]),
            in1=_rd(inv9[0:1, 0:1], [[0, 3], [1, 3], [3, 3]]))
        # x9 into sol[0:9]; d = Bm - x@Am into sol[9:12]
        sol = solve_pool.tile([1, 12], F32, tag="sol", name="sol")
        x9 = sol[0:1, 0:9]
        v.tensor_add(out=x9, in0=_rd(tmp27[0:1, 0:1], [[3, 9]]),
                     in1=_rd(tmp27[0:1, 1:2], [[3, 9]]))
        v.tensor_add(out=x9, in0=x9,
                     in1=_rd(tmp27[0:1, 2:3], [[3, 9]]))

        tmp9 = solve_pool.tile([1, 9], F32, tag="tmp9", name="tmp9")
        v.tensor_mul(
            out=tmp9[0:1, :].rearrange("p (i j) -> p i j", j=3),
            in0=x9.rearrange("p (i j) -> p i j", j=3),
            in1=_rd(Am[0:1, 0:1], [[0, 3], [1, 3]]))
        xAm = solve_pool.tile([1, 3], F32, tag="xAm", name="xAm")
        v.tensor_add(out=xAm[:], in0=_rd(tmp9[0:1, 0:1], [[3, 3]]),
                     in1=_rd(tmp9[0:1, 1:2], [[3, 3]]))
        v.tensor_add(out=xAm[:], in0=xAm[:],
                     in1=_rd(tmp9[0:1, 2:3], [[3, 3]]))
        v.tensor_sub(out=sol[0:1, 9:12], in0=Bm[:], in1=xAm[:])

        xb[s] = solve_pool.tile([P, 12], F32, tag="xb", name="xb")
        nc.gpsimd.partition_broadcast(xb[s][:], sol[0:1, 0:12])

        # all 9 diag(x_ij) weight tiles in one broadcast-AP tensor_mul
        xb16 = solve_pool.tile([P, 12], F16, tag="xb16", name="xb16")
        v.tensor_copy(out=xb16[:], in_=xb[s][:])
        dgall = dg_pool.tile([P, 9 * P], F16, tag="dgall", name="dgall")
        v.tensor_mul(out=_rd(dgall[:, 0:1], [[128, 9], [1, 128]]),
                     in0=_rd(eye16[:, 0:1], [[0, 9], [1, 128]]),
                     in1=_rd(xb16[:, 0:1], [[1, 9], [0, 128]]))
        dg[s] = [[dgall[:, (3 * i + j) * P:(3 * i + j + 1) * P]
                  for j in range(C)] for i in range(C)]
        if s == 0:
            # barrier tile: sample 1's first DVE product writes this tile,
            # so it cannot start (or bypass-run) before the solve finishes
            bar = scr_pool.tile([P, F], F16, tag="scr", name="scr")
            v.tensor_copy(out=bar[0:1, 0:1], in_=dgall[0:1, 0:1])
            barrier_scr[0] = bar

    def emit_pass2_unit(s, g, i, eng=None):
        if eng is None:
            stage = stage_pool.tile([P, ST_W], F16, tag="stage", name="stage")
        elif eng is nc.vector:
            stage = dstage_pool.tile([P, ST_W], F16, tag="dstage",
                                     name="dstage")
        else:
            stage = pstage_pool.tile([P, ST_W], F16, tag="pstage",
                                     name="pstage")
        ga = slice(g * ST_W, (g + 1) * ST_W)
        if eng is not None:
            eng.tensor_scalar(
                out=stage[:], in0=a_t[s][0][:, ga],
                scalar1=xb[s][:, 3 * i: 3 * i + 1],
                scalar2=xb[s][:, 9 + i: 10 + i],
                op0=ALU.mult, op1=ALU.add)
            for j in (1, 2):
                eng.scalar_tensor_tensor(
                    out=stage[:], in0=a_t[s][j][:, ga],
                    scalar=xb[s][:, 3 * i + j: 3 * i + j + 1],
                    in1=stage[:], op0=ALU.mult, op1=ALU.add)
        else:
            for seg in range(ST_W // PS_OUT_W):
                pt = ps_out.tile([P, PS_OUT_W], F32, tag="pt", name="pt")
                o0 = g * ST_W + seg * PS_OUT_W
                for j in range(C):
                    nc.tensor.matmul(
                        pt[:], dg[s][i][j][:],
                        a_t[s][j][:, o0:o0 + PS_OUT_W],
                        start=(j == 0), stop=(j == 2))
                nc.scalar.add(
                    out=stage[:, seg * PS_OUT_W:(seg + 1) * PS_OUT_W],
                    in_=pt[:], add=xb[s][:, 9 + i:10 + i])
        nc.sync.dma_start(out=outv[s, i][:, g * ST_W:(g + 1) * ST_W],
                          in_=stage[:])

    def pass2_units(s):
        units = [(g, i) for g in range(F // ST_W) for i in range(C)]
        dve_set = set(units[-N_P2_DVE:]) if (s == 1 and N_P2_DVE) else set()
        return units, dve_set

    # ---------------- global emission order ----------------
    emit_loads(0)
    emit_loads_a(1)
    emit_loads_b(1, 0, 2)
    emit_alloc_pass1(0)
    emit_alloc_pass1(1)
    emit_act_squares(0)
    emit_pe_aad(0)
    for (c, j) in ba_gram_list(0):
        emit_pe_ba_gram(0, c, j)
    emit_dve_products(0)       # drains s0 extracts between product pairs
    emit_pe_aad(1)             # fills the PE window while solve(0) runs
    emit_act_squares(1)
    emit_extracts(0)
    emit_stats(0)
    emit_solve(0)
    emit_loads_b(1, 2, NQ)
    drain_extracts(1, 1)       # s1 AAX extract; ready before products
    emit_dve_products(1)       # barrier tile gates this behind solve(0)

    # s1-BA grams interleaved with s0 pass2 units on the PE queue
    units0 = [(g, i) for g in range(F // ST_W) for i in range(C)]
    ba1 = ba_gram_list(1)
    ui = 0
    per = max(1, len(units0) // max(1, len(ba1)))
    for n, (c, j) in enumerate(ba1):
        emit_pe_ba_gram(1, c, j)
        if n > 0:
            drain_extracts(1, 1)   # extract of the previous gram, pipelined
        take = per if n < len(ba1) - 1 else len(units0) - ui
        for _ in range(take):
            if ui < len(units0):
                g, i = units0[ui]
                emit_pass2_unit(0, g, i)
                ui += 1

    emit_extracts(1)
    emit_stats(1)
    emit_solve(1)
    units1 = [(g, i) for g in range(F // ST_W) for i in range(C)]
    dve_pos = set()
    if N_P2_DVE:
        step = max(1, len(units1) // N_P2_DVE)
        k = 0
        while len(dve_pos) < N_P2_DVE and k < len(units1):
            dve_pos.add(k)
            k += step
    for idx, (g, i) in enumerate(units1):
        emit_pass2_unit(1, g, i, eng=nc.vector if idx in dve_pos else None)


def build_nc() -> "bass.Bass":
    nc = bacc.Bacc("TRN2", target_bir_lowering=False)
    src = nc.dram_tensor("src", [B_CORE, C, H, W], F32, kind="ExternalInput")
    dst = nc.dram_tensor("dst", [B_CORE, C, H, W], F32, kind="ExternalInput")
    out = nc.dram_tensor("out", [B_CORE, C, H, W], F16, kind="ExternalOutput")
    with tile.TileContext(nc) as tc:
        _colornorm(tc, src[:], dst[:], out[:])
    nc.finalize()
    return nc


_NC = None


def _get_nc():
    global _NC
    if _NC is None:
        _NC = build_nc()
    return _NC


TRACE = False
LAST_RESULT = None  # BassKernelResults of the most recent run (for profiling)


def kernel(src, dst):
    from concourse.bass_utils import run_bass_kernel_spmd

    global LAST_RESULT
    src = np.ascontiguousarray(np.asarray(src, dtype=np.float32))
    dst = np.ascontiguousarray(np.asarray(dst, dtype=np.float32))
    assert src.shape == (NCORES * B_CORE, C, H, W), src.shape
    nc = _get_nc()
    in_maps = [
        {
            "src": np.ascontiguousarray(src[i * B_CORE:(i + 1) * B_CORE]),
            "dst": np.ascontiguousarray(dst[i * B_CORE:(i + 1) * B_CORE]),
        }
        for i in range(NCORES)
    ]
    res = run_bass_kernel_spmd(nc, in_maps, core_ids=list(range(NCORES)),
                               trace=TRACE)
    LAST_RESULT = res
    return np.concatenate(
        [np.asarray(r["out"]).astype(np.float32) for r in res.results], axis=0)
